# revision 26
# baseline (speedup 1.0000x reference)
import sys
import numpy as np

sys.path.insert(0, "/opt/trn_rl_repo")
sys.path.insert(0, "/opt/trn_rl_repo/concourse")

import ml_dtypes
import concourse.bass as bass
import concourse.bacc as bacc
import concourse.mybir as mybir
import concourse.tile as tile
from concourse.bass import IndirectOffsetOnAxis
from concourse.bass_utils import run_bass_kernel_spmd
from concourse.masks import make_identity

F32 = mybir.dt.float32
BF16 = mybir.dt.bfloat16
I32 = mybir.dt.int32
BFNP = ml_dtypes.bfloat16

N = 20000
E = 160000
B = 128
NDEV = 8
NPD = N // NDEV          # 2500 nodes per device
NT = (NPD + 127) // 128  # 20 dst tiles per device
GT = 2                   # tiles per allgather group
NG = NT // GT            # 5 groups
H = 4
C1IN, C1 = 768, 512
C2IN, C2 = 512, 256
CC1 = C1IN // 128        # 6
CC2 = C2IN // 128        # 4
HC1 = H * C1             # 2048
HC2 = H * C2             # 1024
XAW = 800                # [x 768 | as1 4 | ad1 4 | pad] bf16 row (1600B)
TW2 = 1056               # [xw2 1024 | as2 4 | ad2 4 | pad] bf16 row (2112B)
W2C = HC2 + 8            # 1032 cols of [W2 | wa2]
NEG = 0.2


def _host_prep(edge_index, batch):
    """Integer-only preprocessing: edge partitioning, sorting, chunk layout."""
    src = np.concatenate([edge_index[0], np.arange(N, dtype=np.int64)]).astype(np.int64)
    dst = np.concatenate([edge_index[1], np.arange(N, dtype=np.int64)]).astype(np.int64)
    order = np.argsort(dst, kind="stable")
    src, dst = src[order], dst[order]

    dev = dst // NPD
    tloc = (dst % NPD) // 128
    cnt = np.zeros((NDEV, NT), dtype=np.int64)
    for d in range(NDEV):
        m = dev == d
        cnt[d] = np.bincount(tloc[m], minlength=NT)
    Ks = [max(1, int(np.ceil(cnt[:, t].max() / 128.0))) for t in range(NT)]
    SK = sum(Ks)
    offs = np.cumsum([0] + Ks)

    # t2_full row index for source node s (grouped allgather layout)
    s_dev = src // NPD
    s_loc = src % NPD
    s_tl = s_loc // 128
    s_r = s_loc % 128
    t2row = (s_tl // GT) * (NDEV * GT * 128) + s_dev * (GT * 128) + (s_tl % GT) * 128 + s_r

    xidx = np.zeros((NDEV, 128, SK), dtype=np.int32)   # into xa rows
    x2idx = np.zeros((NDEV, 128, SK), dtype=np.int32)  # into t2_full rows
    dstf = np.full((NDEV, 128, SK), -1.0, dtype=np.float32)
    dstfR = np.full((NDEV, SK, 128), -1.0, dtype=np.float32)

    for d in range(NDEV):
        m = dev == d
        s_d, t_d, dl_d, r2_d = src[m], tloc[m], (dst[m] % NPD) % 128, t2row[m]
        for t in range(NT):
            mt = t_d == t
            s_t = s_d[mt]
            dl_t = dl_d[mt]
            r2_t = r2_d[mt]
            o = offs[t]
            j = np.arange(len(s_t))
            xidx[d, j % 128, o + j // 128] = s_t
            x2idx[d, j % 128, o + j // 128] = r2_t
            dstf[d, j % 128, o + j // 128] = dl_t.astype(np.float32)
            dstfR[d, o + j // 128, j % 128] = dl_t.astype(np.float32)

    iota = np.arange(128, dtype=np.float32)
    selN = (dstf[:, :, :, None] == iota).astype(BFNP).reshape(NDEV, 128, SK * 128)
    selTN = (iota[None, :, None, None] == dstfR[:, None, :, :]).astype(BFNP)
    selTN = selTN.reshape(NDEV, 128, SK * 128)

    batchf = np.full((NDEV, 128, NT), -1.0, dtype=np.float32)
    b_np = np.asarray(batch).astype(np.int64)
    for d in range(NDEV):
        for t in range(NT):
            rows = min(128, NPD - t * 128)
            g = b_np[d * NPD + t * 128: d * NPD + t * 128 + rows]
            batchf[d, :rows, t] = g.astype(np.float32)

    return Ks, offs, SK, xidx, x2idx, selN, selTN, batchf


def _build(Ks, offs, SK):
    """Emit the Bass program (identical for all 8 cores)."""
    nc = bacc.Bacc("TRN2", target_bir_lowering=False, debug=False, num_devices=NDEV)

    # ---- I/O ----
    xa_t = nc.dram_tensor("xa", [N, XAW], BF16, kind="ExternalInput")
    xlocT_t = nc.dram_tensor("xlocT", [C1IN, NPD], BF16, kind="ExternalInput")
    W1_t = nc.dram_tensor("W1", [C1IN, HC1], BF16, kind="ExternalInput")
    W2c_t = nc.dram_tensor("W2c", [C2IN, W2C], BF16, kind="ExternalInput")
    wa1_t = nc.dram_tensor("wa1", [C1IN, 8], BF16, kind="ExternalInput")
    b1_t = nc.dram_tensor("b1", [C1], BF16, kind="ExternalInput")
    b2_t = nc.dram_tensor("b2", [C2], F32, kind="ExternalInput")
    fcW_t = nc.dram_tensor("fcW", [C2, 2], F32, kind="ExternalInput")
    fcb_t = nc.dram_tensor("fcb", [2], F32, kind="ExternalInput")
    xidx_t = nc.dram_tensor("xidx", [128, SK], I32, kind="ExternalInput")
    x2idx_t = nc.dram_tensor("x2idx", [128, SK], I32, kind="ExternalInput")
    selN_t = nc.dram_tensor("selN", [128, SK * 128], BF16, kind="ExternalInput")
    selTN_t = nc.dram_tensor("selTN", [128, SK * 128], BF16, kind="ExternalInput")
    batchf_t = nc.dram_tensor("batchf", [128, NT], F32, kind="ExternalInput")
    y_t = nc.dram_tensor("y", [B, 2], F32, kind="ExternalOutput")

    # ---- internal DRAM ----
    asad1_loc = nc.dram_tensor("asad1_loc", [NPD, 8], BF16)
    asad1_full = nc.dram_tensor("asad1_full", [N, 8], BF16, addr_space="Shared")
    t2_loc = nc.dram_tensor("t2_loc", [NT * 128, TW2], BF16)
    t2_full = nc.dram_tensor("t2_full", [NG * NDEV * GT * 128, TW2], BF16,
                             addr_space="Shared")
    rdscr = nc.dram_tensor("rdscr", [NT, 512], F32)
    pc_loc = nc.dram_tensor("pc_loc", [B, C2 + 1], F32)
    pc_red = nc.dram_tensor("pc_red", [B, C2 + 1], F32, addr_space="Shared")

    RG = [list(range(NDEV))]
    KMAX = max(Ks)

    import os as _os
    DEBUG = bool(int(_os.environ.get("BASS_GAT_DEBUG", "0")))
    if DEBUG:
        dbg_h1 = nc.dram_tensor("dbg_h1", [NT * 128, C1], F32, kind="ExternalOutput")
        dbg_t2 = nc.dram_tensor("dbg_t2", [NG * NDEV * GT * 128, TW2], F32,
                                kind="ExternalOutput")
        dbg_pc = nc.dram_tensor("dbg_pc", [B, C2 + 1], F32, kind="ExternalOutput")

    with tile.TileContext(nc) as tc:
        with (
            tc.tile_pool(name="const", bufs=1) as cp,
            tc.tile_pool(name="small", bufs=5) as sp,
            tc.tile_pool(name="selp", bufs=4) as selp,
            tc.tile_pool(name="selwp", bufs=4) as selwp,
            tc.tile_pool(name="xgp", bufs=2) as xgp,
            tc.tile_pool(name="hgp", bufs=2) as hgp,
            tc.tile_pool(name="dstp", bufs=2) as dstp,
            tc.tile_pool(name="utp", bufs=2) as utp,
            tc.tile_pool(name="t2p", bufs=2) as t2p,
        ):
            # ================= constants =================
            ident = cp.tile([128, 128], F32, tag="ident")
            make_identity(nc, ident[:])
            identb = cp.tile([128, 128], BF16, tag="identb")
            make_identity(nc, identb[:])
            iota_i = cp.tile([128, 128], I32, tag="iota_i")
            nc.gpsimd.iota(iota_i[:], pattern=[[1, 128]], base=0, channel_multiplier=0)
            iotaT = cp.tile([128, 128], F32, tag="iotaT")
            nc.vector.tensor_copy(out=iotaT[:], in_=iota_i[:])
            iota_ci = cp.tile([128, 1], I32, tag="iota_ci")
            nc.gpsimd.iota(iota_ci[:], pattern=[[1, 1]], base=0, channel_multiplier=1)
            iotaC = cp.tile([128, 1], F32, tag="iotaC")
            nc.vector.tensor_copy(out=iotaC[:], in_=iota_ci[:])
            ones1 = cp.tile([1, 128], BF16, tag="ones1")
            nc.vector.memset(ones1[:], 1.0)
            ones1f = cp.tile([1, 128], F32, tag="ones1f")
            nc.vector.memset(ones1f[:], 1.0)

            b1_sb = cp.tile([1, C1], BF16, tag="b1")
            nc.scalar.dma_start(out=b1_sb[:], in_=b1_t[None, :])
            b2bc = cp.tile([128, C2], F32, tag="b2bc")
            nc.scalar.dma_start(out=b2bc[:], in_=b2_t[None, :].to_broadcast([128, C2]))
            fcb_sb = cp.tile([1, 2], F32, tag="fcb")
            nc.scalar.dma_start(out=fcb_sb[:], in_=fcb_t[None, :])
            fcW_sb = cp.tile([128, 4], F32, tag="fcW")
            for c in range(2):
                nc.scalar.dma_start(out=fcW_sb[:, 2 * c:2 * c + 2],
                                  in_=fcW_t[c * 128:(c + 1) * 128, :])

            W1_sb = cp.tile([128, CC1 * HC1], BF16, tag="W1")
            for c in range(CC1):
                nc.scalar.dma_start(out=W1_sb[:, c * HC1:(c + 1) * HC1],
                                  in_=W1_t[c * 128:(c + 1) * 128, :])
            W2c_sb = cp.tile([128, CC2 * W2C], BF16, tag="W2c")
            for c in range(CC2):
                nc.scalar.dma_start(out=W2c_sb[:, c * W2C:(c + 1) * W2C],
                                  in_=W2c_t[c * 128:(c + 1) * 128, :])
            wa1_sb = cp.tile([128, CC1 * 8], BF16, tag="wa1")
            nc.scalar.dma_start(
                out=wa1_sb[:].rearrange("p (c j) -> p c j", j=8),
                in_=wa1_t[:].rearrange("(c p) j -> p c j", p=128))
            batchf_sb = cp.tile([128, NT], F32, tag="batchf")
            nc.scalar.dma_start(out=batchf_sb[:], in_=batchf_t[:, :])
            poolacc = cp.tile([128, C2 + 1], F32, tag="poolacc")
            nc.vector.memset(poolacc[:], 0.0)

            # ============ asad1 = x_loc @ wa1 ============
            with (
                tc.tile_pool(name="prep", bufs=2) as pp,
                tc.tile_pool(name="pshp", bufs=2, space="PSUM") as pshp,
            ):
                for t in range(NT):
                    rows = min(128, NPD - t * 128)
                    xT = pp.tile([128, CC1 * 128], BF16, tag="xT")
                    nc.sync.dma_start(
                        out=xT[:, :CC1 * rows].rearrange("p (c n) -> p c n", c=CC1),
                        in_=xlocT_t[:, t * 128: t * 128 + rows].rearrange(
                            "(c p) n -> p c n", p=128))
                    ps = pshp.tile([128, 8], F32, tag="ps")
                    for c in range(CC1):
                        nc.tensor.matmul(out=ps[:rows, :],
                                         lhsT=xT[:, c * rows:(c + 1) * rows],
                                         rhs=wa1_sb[:, c * 8:(c + 1) * 8],
                                         start=(c == 0), stop=(c == CC1 - 1))
                    as1 = pp.tile([128, 8], BF16, tag="as1")
                    nc.vector.tensor_copy(out=as1[:rows, :], in_=ps[:rows, :])
                    nc.sync.dma_start(out=asad1_loc[t * 128: t * 128 + rows, :],
                                      in_=as1[:rows, :])

            nc.gpsimd.collective_compute(
                "AllGather", mybir.AluOpType.bypass, replica_groups=RG,
                ins=[asad1_loc[:, :]], outs=[asad1_full[:, :]])
            NW = 12
            CH = (N + NW - 1) // NW
            for w in range(NW):
                lo, hi = w * CH, min(N, (w + 1) * CH)
                q = nc.sync
                q.dma_start(out=xa_t[lo:hi, C1IN:C1IN + 8],
                            in_=asad1_full[lo:hi, :])

            # ================= layer 1 sweep =================
            with (
                tc.tile_pool(name="psu", bufs=1, space="PSUM") as psu,
                tc.tile_pool(name="psh", bufs=1, space="PSUM") as psh,
                tc.tile_pool(name="psr", bufs=1, space="PSUM") as psr,
            ):
                for t in range(NT):
                    K = Ks[t]
                    o = offs[t]
                    rows = min(128, NPD - t * 128)

                    idx_sb = sp.tile([128, KMAX], I32, tag="idx")
                    nc.scalar.dma_start(out=idx_sb[:, :K], in_=xidx_t[:, o:o + K])
                    ad_sb = sp.tile([128, 4], BF16, tag="ad")
                    if rows < 128:
                        nc.vector.memset(ad_sb[:], 0.0)
                    nc.scalar.dma_start(out=ad_sb[:rows, :],
                                      in_=asad1_loc[t * 128:t * 128 + rows, 4:8])

                    # ---------- gather x rows (asad embedded) ----------
                    xg = xgp.tile([128, KMAX * XAW], BF16, tag="xg")
                    for k in range(K):
                        nc.gpsimd.indirect_dma_start(
                            out=xg[:, k * XAW:(k + 1) * XAW], out_offset=None,
                            in_=xa_t[:, :],
                            in_offset=IndirectOffsetOnAxis(
                                ap=idx_sb[:, k:k + 1], axis=0))
                    xg3 = xg[:, :K * XAW].rearrange("p (k d) -> p k d", d=XAW)

                    # ---------- selection matrices (batched) ----------
                    selT = selp.tile([128, KMAX * 128], BF16, tag="selT")
                    nc.scalar.dma_start(out=selT[:, :K * 128],
                                        in_=selTN_t[:, o * 128:(o + K) * 128])
                    sel = selp.tile([128, KMAX * 128], BF16, tag="sel")
                    nc.scalar.dma_start(out=sel[:, :K * 128],
                                        in_=selN_t[:, o * 128:(o + K) * 128])

                    # ---------- e = lrelu(as+ad); ex = exp ----------
                    ade_ps = psh.tile([128, 4 * KMAX], F32, tag="hold")
                    for k in range(K):
                        nc.tensor.matmul(out=ade_ps[:, 4 * k:4 * k + 4],
                                         lhsT=selT[:, k * 128:(k + 1) * 128],
                                         rhs=ad_sb[:],
                                         start=(k == 0), stop=(k == K - 1))
                    asg = sp.tile([128, 4 * KMAX], F32, tag="asg")
                    nc.vector.tensor_copy(out=asg[:, :4 * K].rearrange(
                        "p (k s) -> p k s", s=4),
                        in_=xg3[:, :, C1IN:C1IN + 4])
                    z = sp.tile([128, 4 * KMAX], F32, tag="z")
                    nc.vector.tensor_tensor(out=z[:, :4 * K], in0=ade_ps[:, :4 * K],
                                            in1=asg[:, :4 * K],
                                            op=mybir.AluOpType.add)
                    zs = sp.tile([128, 4 * KMAX], F32, tag="zs")
                    nc.vector.tensor_scalar_mul(zs[:, :4 * K], z[:, :4 * K], NEG)
                    nc.vector.tensor_tensor(out=z[:, :4 * K], in0=z[:, :4 * K],
                                            in1=zs[:, :4 * K],
                                            op=mybir.AluOpType.max)
                    exf = sp.tile([128, 4 * KMAX], F32, tag="exf")
                    nc.scalar.activation(out=exf[:, :4 * K], in_=z[:, :4 * K],
                                         func=mybir.ActivationFunctionType.Exp)
                    ex = sp.tile([128, 4 * KMAX], BF16, tag="ex")
                    nc.vector.tensor_copy(out=ex[:, :4 * K], in_=exf[:, :4 * K])

                    # ---------- denT[h, d] = sum_e ex[e,h] sel[e,d] ----------
                    denT_ps = psh.tile([4, 128], F32, tag="hold")
                    for k in range(K):
                        nc.tensor.matmul(out=denT_ps[:],
                                         lhsT=ex[:, 4 * k:4 * k + 4],
                                         rhs=sel[:, k * 128:(k + 1) * 128],
                                         start=(k == 0), stop=(k == K - 1))

                    # ---------- aggregate ut[f,(h,d)] += x ex ----------
                    ut_ps = [psu.tile([128, 512], F32, tag=f"ut{c}",
                                      name=f"ut_ps{c}")
                             for c in range(CC1)]
                    for k in range(K):
                        selw = selwp.tile([128, 512], BF16, tag="selw")
                        for h in range(H):
                            nc.vector.tensor_scalar_mul(
                                selw[:, h * 128:(h + 1) * 128],
                                sel[:, k * 128:(k + 1) * 128],
                                exf[:, 4 * k + h:4 * k + h + 1])
                        for c in range(CC1):
                            nc.tensor.matmul(
                                out=ut_ps[c][:],
                                lhsT=xg[:, k * XAW + c * 128: k * XAW + (c + 1) * 128],
                                rhs=selw[:],
                                start=(k == 0), stop=(k == K - 1))

                    # ---------- rdenT broadcast + divide ----------
                    rdT = sp.tile([4, 128], F32, tag="rdT")
                    nc.vector.tensor_scalar(out=rdT[:], in0=denT_ps[:],
                                            scalar1=4.0, scalar2=1e-16,
                                            op0=mybir.AluOpType.mult,
                                            op1=mybir.AluOpType.max)
                    nc.vector.reciprocal(out=rdT[:], in_=rdT[:])
                    nc.sync.dma_start(
                        out=rdscr[t:t + 1, :].rearrange("a (p f) -> (a p) f", p=4),
                        in_=rdT[:])
                    rdb = dstp.tile([128, 512], F32, tag="rdb")
                    nc.sync.dma_start(out=rdb[:],
                                      in_=rdscr[t:t + 1, :].to_broadcast([128, 512]))
                    ut_sb = utp.tile([128, CC1 * 512], BF16, tag="ut")
                    for c in range(CC1):
                        nc.vector.tensor_tensor(out=ut_sb[:, c * 512:(c + 1) * 512],
                                                in0=ut_ps[c][:],
                                                in1=rdb[:],
                                                op=mybir.AluOpType.mult)

                    # ---------- project + bias ----------
                    out_ps = psh.tile([128, C1], F32, tag="hold")
                    first = True
                    for c in range(CC1):
                        for h in range(H):
                            nc.tensor.matmul(
                                out=out_ps[:],
                                lhsT=ut_sb[:, c * 512 + h * 128: c * 512 + (h + 1) * 128],
                                rhs=W1_sb[:, c * HC1 + h * C1: c * HC1 + (h + 1) * C1],
                                start=first, stop=False)
                            first = False
                    nc.tensor.matmul(out=out_ps[:], lhsT=ones1[:], rhs=b1_sb[:],
                                     start=False, stop=True)
                    h1_sb = t2p.tile([128, C1], BF16, tag="h1")
                    nc.vector.tensor_copy(out=h1_sb[:], in_=out_ps[:])
                    if DEBUG:
                        h1f = sp.tile([128, C1], F32, tag="h1f")
                        nc.vector.tensor_copy(out=h1f[:], in_=out_ps[:])
                        nc.sync.dma_start(out=dbg_h1[t * 128:(t + 1) * 128, :],
                                          in_=h1f[:])

                    # ---------- xw2 = h1 @ [W2 | wa2] ----------
                    h1T = t2p.tile([128, C1], BF16, tag="h1T")
                    for c in range(CC2):
                        tp = psr.tile([128, 128], BF16, tag="rot")
                        nc.tensor.transpose(out=tp[:],
                                            in_=h1_sb[:, c * 128:(c + 1) * 128],
                                            identity=identb[:])
                        nc.vector.tensor_copy(out=h1T[:, c * 128:(c + 1) * 128],
                                              in_=tp[:])
                    xw2_sb = t2p.tile([128, TW2], BF16, tag="xw2")
                    for (lo, w) in ((0, 512), (512, 512), (1024, 8)):
                        xp = psr.tile([128, 512], F32, tag="rot")
                        for c in range(CC2):
                            nc.tensor.matmul(
                                out=xp[:, :w],
                                lhsT=h1T[:, c * 128:(c + 1) * 128],
                                rhs=W2c_sb[:, c * W2C + lo: c * W2C + lo + w],
                                start=(c == 0), stop=(c == CC2 - 1))
                        nc.vector.tensor_copy(out=xw2_sb[:, lo:lo + w], in_=xp[:, :w])
                    nc.sync.dma_start(out=t2_loc[t * 128:(t + 1) * 128, :],
                                      in_=xw2_sb[:])

                    if t % GT == GT - 1:
                        g = t // GT
                        nc.gpsimd.collective_compute(
                            "AllGather", mybir.AluOpType.bypass, replica_groups=RG,
                            ins=[t2_loc[g * GT * 128:(g + 1) * GT * 128, :]],
                            outs=[t2_full[g * NDEV * GT * 128:
                                          (g + 1) * NDEV * GT * 128, :]])

            if DEBUG:
                t2f = sp.tile([128, TW2], F32, tag="t2f")
                for i in range(NG * NDEV * GT):
                    t2b = sp.tile([128, TW2], BF16, tag="t2b")
                    nc.sync.dma_start(out=t2b[:],
                                      in_=t2_full[i * 128:(i + 1) * 128, :])
                    nc.vector.tensor_copy(out=t2f[:], in_=t2b[:])
                    nc.sync.dma_start(out=dbg_t2[i * 128:(i + 1) * 128, :],
                                      in_=t2f[:])

            # ================= layer 2 sweep + pooling =================
            with (
                tc.tile_pool(name="psu2", bufs=1, space="PSUM") as psu2,
                tc.tile_pool(name="psh2", bufs=2, space="PSUM") as psh2,
            ):
                for t in range(NT):
                    K = Ks[t]
                    o = offs[t]
                    rows = min(128, NPD - t * 128)

                    idx_sb = sp.tile([128, KMAX], I32, tag="idx")
                    nc.scalar.dma_start(out=idx_sb[:, :K], in_=x2idx_t[:, o:o + K])
                    ad_sb = sp.tile([128, 4], BF16, tag="ad")
                    if rows < 128:
                        nc.vector.memset(ad_sb[:], 0.0)
                    nc.scalar.dma_start(out=ad_sb[:rows, :],
                                      in_=t2_loc[t * 128:t * 128 + rows,
                                                 HC2 + 4:HC2 + 8])

                    hg = hgp.tile([128, KMAX * TW2], BF16, tag="hg")
                    for k in range(K):
                        nc.gpsimd.indirect_dma_start(
                            out=hg[:, k * TW2:(k + 1) * TW2], out_offset=None,
                            in_=t2_full[:, :],
                            in_offset=IndirectOffsetOnAxis(
                                ap=idx_sb[:, k:k + 1], axis=0))
                    hg3 = hg[:, :K * TW2].rearrange("p (k d) -> p k d", d=TW2)

                    selT = selp.tile([128, KMAX * 128], BF16, tag="selT")
                    nc.scalar.dma_start(out=selT[:, :K * 128],
                                        in_=selTN_t[:, o * 128:(o + K) * 128])
                    sel = selp.tile([128, KMAX * 128], BF16, tag="sel")
                    nc.scalar.dma_start(out=sel[:, :K * 128],
                                        in_=selN_t[:, o * 128:(o + K) * 128])

                    ade_ps = psh2.tile([128, 4 * KMAX], F32, tag="hold")
                    for k in range(K):
                        nc.tensor.matmul(out=ade_ps[:, 4 * k:4 * k + 4],
                                         lhsT=selT[:, k * 128:(k + 1) * 128],
                                         rhs=ad_sb[:],
                                         start=(k == 0), stop=(k == K - 1))
                    asg = sp.tile([128, 4 * KMAX], F32, tag="asg")
                    nc.vector.tensor_copy(out=asg[:, :4 * K].rearrange(
                        "p (k s) -> p k s", s=4),
                        in_=hg3[:, :, HC2:HC2 + 4])
                    z = sp.tile([128, 4 * KMAX], F32, tag="z")
                    nc.vector.tensor_tensor(out=z[:, :4 * K], in0=ade_ps[:, :4 * K],
                                            in1=asg[:, :4 * K],
                                            op=mybir.AluOpType.add)
                    zs = sp.tile([128, 4 * KMAX], F32, tag="zs")
                    nc.vector.tensor_scalar_mul(zs[:, :4 * K], z[:, :4 * K], NEG)
                    nc.vector.tensor_tensor(out=z[:, :4 * K], in0=z[:, :4 * K],
                                            in1=zs[:, :4 * K],
                                            op=mybir.AluOpType.max)
                    exf = sp.tile([128, 4 * KMAX], F32, tag="exf")
                    nc.scalar.activation(out=exf[:, :4 * K], in_=z[:, :4 * K],
                                         func=mybir.ActivationFunctionType.Exp)
                    ex = sp.tile([128, 4 * KMAX], BF16, tag="ex")
                    nc.vector.tensor_copy(out=ex[:, :4 * K], in_=exf[:, :4 * K])

                    # agg[d, (h,c)] += ex * xw2 ; den[d, h] += ex
                    agg_ps = psu2.tile([128, 4 * 512], F32, tag="ut2")
                    den_ps = psh2.tile([128, 4], F32, tag="hold")
                    for k in range(K):
                        selh = selwp.tile([128, 512], BF16, tag="selh")
                        for h in range(H):
                            nc.vector.tensor_scalar_mul(
                                selh[:, h * 128:(h + 1) * 128],
                                sel[:, k * 128:(k + 1) * 128],
                                exf[:, 4 * k + h:4 * k + h + 1])
                        for h in range(H):
                            nc.tensor.matmul(
                                out=agg_ps[:, h * 512:h * 512 + C2],
                                lhsT=selh[:, h * 128:(h + 1) * 128],
                                rhs=hg[:, k * TW2 + h * C2: k * TW2 + (h + 1) * C2],
                                start=(k == 0), stop=(k == K - 1))
                        nc.tensor.matmul(out=den_ps[:],
                                         lhsT=sel[:, k * 128:(k + 1) * 128],
                                         rhs=ex[:, 4 * k:4 * k + 4],
                                         start=(k == 0), stop=(k == K - 1))

                    rd2 = sp.tile([128, 4], F32, tag="rd2")
                    nc.vector.tensor_scalar(out=rd2[:], in0=den_ps[:],
                                            scalar1=4.0, scalar2=1e-16,
                                            op0=mybir.AluOpType.mult,
                                            op1=mybir.AluOpType.max)
                    nc.vector.reciprocal(out=rd2[:], in_=rd2[:])
                    h2acc = sp.tile([128, C2], F32, tag="h2acc")
                    nc.vector.tensor_scalar_mul(h2acc[:], agg_ps[:, 0:C2],
                                                rd2[:, 0:1])
                    for h in range(1, H):
                        nc.vector.scalar_tensor_tensor(
                            out=h2acc[:], in0=agg_ps[:, h * 512:h * 512 + C2],
                            scalar=rd2[:, h:h + 1], in1=h2acc[:],
                            op0=mybir.AluOpType.mult, op1=mybir.AluOpType.add)
                    h2p = t2p.tile([128, C2 + 1], BF16, tag="h2p")
                    nc.vector.tensor_tensor(out=h2p[:, :C2], in0=h2acc[:],
                                            in1=b2bc[:], op=mybir.AluOpType.add)
                    nc.vector.memset(h2p[:, C2:C2 + 1], 1.0)

                    selB = sp.tile([128, 128], BF16, tag="selB")
                    nc.vector.tensor_tensor(
                        out=selB[:],
                        in0=batchf_sb[:, t:t + 1].to_broadcast([128, 128]),
                        in1=iotaT[:], op=mybir.AluOpType.is_equal)
                    pc_ps = psh2.tile([128, C2 + 1], F32, tag="hold")
                    nc.tensor.matmul(out=pc_ps[:], lhsT=selB[:], rhs=h2p[:],
                                     start=True, stop=True)
                    nc.vector.tensor_tensor(out=poolacc[:], in0=poolacc[:],
                                            in1=pc_ps[:], op=mybir.AluOpType.add)

                # ================= pool reduce + FC =================
                nc.sync.dma_start(out=pc_loc[:, :], in_=poolacc[:])
                nc.gpsimd.collective_compute(
                    "AllReduce", mybir.AluOpType.add, replica_groups=RG,
                    ins=[pc_loc[:, :]], outs=[pc_red[:, :]])
                pc_sb = sp.tile([128, C2 + 1], F32, tag="pc")
                nc.sync.dma_start(out=pc_sb[:], in_=pc_red[:, :])
                if DEBUG:
                    nc.sync.dma_start(out=dbg_pc[:, :], in_=pc_red[:, :])
                cnt = sp.tile([128, 1], F32, tag="cnt")
                nc.vector.tensor_scalar_max(cnt[:], pc_sb[:, C2:C2 + 1], 1.0)
                nc.vector.reciprocal(out=cnt[:], in_=cnt[:])
                g_sb = sp.tile([128, C2], F32, tag="g")
                nc.vector.tensor_scalar_mul(g_sb[:], pc_sb[:, :C2], cnt[:, :1])

                y_ps = psh2.tile([128, 2], F32, tag="hold")
                for c in range(2):
                    tp = psu2.tile([128, 128], F32, tag="ut2")
                    nc.tensor.transpose(out=tp[:], in_=g_sb[:, c * 128:(c + 1) * 128],
                                        identity=ident[:])
                    gT = sp.tile([128, 128], F32, tag="gT")
                    nc.vector.tensor_copy(out=gT[:], in_=tp[:])
                    nc.tensor.matmul(out=y_ps[:], lhsT=gT[:],
                                     rhs=fcW_sb[:, 2 * c:2 * c + 2],
                                     start=(c == 0), stop=False)
                nc.tensor.matmul(out=y_ps[:], lhsT=ones1f[:], rhs=fcb_sb[:],
                                 start=False, stop=True)
                y_sb = sp.tile([128, 2], F32, tag="y")
                nc.vector.tensor_copy(out=y_sb[:], in_=y_ps[:])
                nc.sync.dma_start(out=y_t[:, :], in_=y_sb[:])

    nc.compile()
    return nc


_CACHE = {}


def kernel(**inputs):
    x = np.ascontiguousarray(np.asarray(inputs["x"], dtype=np.float32))
    edge_index = np.asarray(inputs["edge_index"])
    batch = np.asarray(inputs["batch"])
    W1 = np.asarray(inputs["W1"], dtype=np.float32)
    W2 = np.asarray(inputs["W2"], dtype=np.float32)
    a_src1 = np.asarray(inputs["a_src1"], dtype=np.float32)
    a_dst1 = np.asarray(inputs["a_dst1"], dtype=np.float32)
    a_src2 = np.asarray(inputs["a_src2"], dtype=np.float32)
    a_dst2 = np.asarray(inputs["a_dst2"], dtype=np.float32)
    b1 = np.asarray(inputs["b1"], dtype=np.float32)
    b2 = np.asarray(inputs["b2"], dtype=np.float32)
    fcW = np.ascontiguousarray(np.asarray(inputs["fcW"], dtype=np.float32))
    fcb = np.asarray(inputs["fcb"], dtype=np.float32)

    Ks, offs, SK, xidx, x2idx, selN, selTN, batchf = _host_prep(edge_index, batch)

    key = (tuple(Ks),)
    if key not in _CACHE:
        _CACHE[key] = _build(Ks, offs, SK)
    nc = _CACHE[key]

    # weight-only prep: wa = W^T a per head (folded attention projections)
    wa1 = np.zeros((C1IN, 8), dtype=np.float32)
    wa2 = np.zeros((C2IN, 8), dtype=np.float32)
    for h in range(H):
        wa1[:, h] = W1[:, h * C1:(h + 1) * C1] @ a_src1[h]
        wa1[:, 4 + h] = W1[:, h * C1:(h + 1) * C1] @ a_dst1[h]
        wa2[:, h] = W2[:, h * C2:(h + 1) * C2] @ a_src2[h]
        wa2[:, 4 + h] = W2[:, h * C2:(h + 1) * C2] @ a_dst2[h]
    W2c = np.concatenate([W2, wa2], axis=1)

    xa = np.zeros((N, XAW), dtype=BFNP)
    xa[:, :C1IN] = x.astype(BFNP)

    in_maps = []
    for d in range(NDEV):
        xloc = x[d * NPD:(d + 1) * NPD]
        in_maps.append({
            "xa": xa,
            "xlocT": np.ascontiguousarray(xloc.T).astype(BFNP),
            "W1": W1.astype(BFNP), "W2c": W2c.astype(BFNP),
            "wa1": wa1.astype(BFNP), "b1": b1.astype(BFNP), "b2": b2,
            "fcW": fcW, "fcb": fcb,
            "xidx": xidx[d], "x2idx": x2idx[d], "selN": selN[d],
            "selTN": selTN[d], "batchf": batchf[d],
        })

    import os as _os
    trace = bool(int(_os.environ.get("BASS_GAT_TRACE", "0")))
    kwargs = {}
    if trace:
        _setup_ntff_hook()
        kwargs = dict(trace=True, trace_cores=[0])
    res = run_bass_kernel_spmd(nc, in_maps, core_ids=list(range(NDEV)), **kwargs)
    if trace:
        kernel.last_exec_ns = res.exec_time_ns
        kernel.last_trace = res.instructions_and_trace
        if res.exec_time_ns is not None:
            print(f"HW exec time: {res.exec_time_ns} ns")
    if bool(int(_os.environ.get("BASS_GAT_DEBUG", "0"))):
        kernel.debug_results = res.results
    return res.results[0]["y"]


def _setup_ntff_hook():
    """The image's antenv lacks axon_hooks; synthesize it and register the
    ctypes NTFF profiling hook so trace=True works."""
    import types
    import antenv
    if hasattr(antenv, "axon_hooks"):
        return
    mod = types.ModuleType("antenv.axon_hooks")
    state = {"hook": None}
    mod.set_axon_ntff_profile_hook = lambda h: state.__setitem__("hook", h)
    mod.get_axon_ntff_profile_hook = lambda: state["hook"]
    sys.modules["antenv.axon_hooks"] = mod
    antenv.axon_hooks = mod
    try:
        from trn_agent_boot.trn_boot import _ntff_profile_via_ctypes
        hook = _ntff_profile_via_ctypes("/opt/axon/libaxon_pjrt.so")
        mod.set_axon_ntff_profile_hook(hook)
    except Exception as e:
        print("ntff hook setup failed:", e)


# revision 27
# speedup vs baseline: 1.0758x; 1.0758x over previous
import sys
import numpy as np

sys.path.insert(0, "/opt/trn_rl_repo")
sys.path.insert(0, "/opt/trn_rl_repo/concourse")

import ml_dtypes
import concourse.bass as bass
import concourse.bacc as bacc
import concourse.mybir as mybir
import concourse.tile as tile
from concourse.bass import IndirectOffsetOnAxis
from concourse.bass_utils import run_bass_kernel_spmd
from concourse.masks import make_identity

F32 = mybir.dt.float32
BF16 = mybir.dt.bfloat16
I32 = mybir.dt.int32
BFNP = ml_dtypes.bfloat16

N = 20000
E = 160000
B = 128
NDEV = 8
NPD = N // NDEV          # 2500 nodes per device
NT = (NPD + 127) // 128  # 20 dst tiles per device
GT = 2                   # tiles per allgather group
NG = NT // GT            # 5 groups
H = 4
C1IN, C1 = 768, 512
C2IN, C2 = 512, 256
CC1 = C1IN // 128        # 6
CC2 = C2IN // 128        # 4
HC1 = H * C1             # 2048
HC2 = H * C2             # 1024
XAW = 800                # [x 768 | as1 4 | ad1 4 | pad] bf16 row (1600B)
TW2 = 1056               # [xw2 1024 | as2 4 | ad2 4 | pad] bf16 row (2112B)
W2C = HC2 + 8            # 1032 cols of [W2 | wa2]
NEG = 0.2


def _host_prep(edge_index, batch):
    """Integer-only preprocessing: edge partitioning, sorting, chunk layout."""
    src = np.concatenate([edge_index[0], np.arange(N, dtype=np.int64)]).astype(np.int64)
    dst = np.concatenate([edge_index[1], np.arange(N, dtype=np.int64)]).astype(np.int64)
    order = np.argsort(dst, kind="stable")
    src, dst = src[order], dst[order]

    dev = dst // NPD
    tloc = (dst % NPD) // 128
    cnt = np.zeros((NDEV, NT), dtype=np.int64)
    for d in range(NDEV):
        m = dev == d
        cnt[d] = np.bincount(tloc[m], minlength=NT)
    Ks = [max(1, int(np.ceil(cnt[:, t].max() / 128.0))) for t in range(NT)]
    SK = sum(Ks)
    offs = np.cumsum([0] + Ks)

    # t2_full row index for source node s (grouped allgather layout)
    s_dev = src // NPD
    s_loc = src % NPD
    s_tl = s_loc // 128
    s_r = s_loc % 128
    t2row = (s_tl // GT) * (NDEV * GT * 128) + s_dev * (GT * 128) + (s_tl % GT) * 128 + s_r

    xidx = np.zeros((NDEV, 128, SK), dtype=np.int32)   # into xa rows
    x2idx = np.zeros((NDEV, 128, SK), dtype=np.int32)  # into t2_full rows
    dstf = np.full((NDEV, 128, SK), -1.0, dtype=np.float32)
    dstfR = np.full((NDEV, SK, 128), -1.0, dtype=np.float32)

    for d in range(NDEV):
        m = dev == d
        s_d, t_d, dl_d, r2_d = src[m], tloc[m], (dst[m] % NPD) % 128, t2row[m]
        for t in range(NT):
            mt = t_d == t
            s_t = s_d[mt]
            dl_t = dl_d[mt]
            r2_t = r2_d[mt]
            o = offs[t]
            j = np.arange(len(s_t))
            xidx[d, j % 128, o + j // 128] = s_t
            x2idx[d, j % 128, o + j // 128] = r2_t
            dstf[d, j % 128, o + j // 128] = dl_t.astype(np.float32)
            dstfR[d, o + j // 128, j % 128] = dl_t.astype(np.float32)

    iota = np.arange(128, dtype=np.float32)
    selN = (dstf[:, :, :, None] == iota).astype(BFNP).reshape(NDEV, 128, SK * 128)
    selTN = (iota[None, :, None, None] == dstfR[:, None, :, :]).astype(BFNP)
    selTN = selTN.reshape(NDEV, 128, SK * 128)

    batchf = np.full((NDEV, 128, NT), -1.0, dtype=np.float32)
    b_np = np.asarray(batch).astype(np.int64)
    for d in range(NDEV):
        for t in range(NT):
            rows = min(128, NPD - t * 128)
            g = b_np[d * NPD + t * 128: d * NPD + t * 128 + rows]
            batchf[d, :rows, t] = g.astype(np.float32)

    return Ks, offs, SK, xidx, x2idx, selN, selTN, batchf


def _build(Ks, offs, SK):
    """Emit the Bass program (identical for all 8 cores)."""
    nc = bacc.Bacc("TRN2", target_bir_lowering=False, debug=False, num_devices=NDEV)

    # ---- I/O ----
    xa_t = nc.dram_tensor("xa", [N, XAW], BF16, kind="ExternalInput")
    xlocT_t = nc.dram_tensor("xlocT", [C1IN, NPD], BF16, kind="ExternalInput")
    W1_t = nc.dram_tensor("W1", [C1IN, HC1], BF16, kind="ExternalInput")
    W2c_t = nc.dram_tensor("W2c", [C2IN, W2C], BF16, kind="ExternalInput")
    wa1_t = nc.dram_tensor("wa1", [C1IN, 8], BF16, kind="ExternalInput")
    b1_t = nc.dram_tensor("b1", [C1], BF16, kind="ExternalInput")
    b2_t = nc.dram_tensor("b2", [C2], F32, kind="ExternalInput")
    fcW_t = nc.dram_tensor("fcW", [C2, 2], F32, kind="ExternalInput")
    fcb_t = nc.dram_tensor("fcb", [2], F32, kind="ExternalInput")
    xidx_t = nc.dram_tensor("xidx", [128, SK], I32, kind="ExternalInput")
    x2idx_t = nc.dram_tensor("x2idx", [128, SK], I32, kind="ExternalInput")
    selN_t = nc.dram_tensor("selN", [128, SK * 128], BF16, kind="ExternalInput")
    selTN_t = nc.dram_tensor("selTN", [128, SK * 128], BF16, kind="ExternalInput")
    batchf_t = nc.dram_tensor("batchf", [128, NT], F32, kind="ExternalInput")
    y_t = nc.dram_tensor("y", [B, 2], F32, kind="ExternalOutput")

    # ---- internal DRAM ----
    asad1_loc = nc.dram_tensor("asad1_loc", [NPD, 8], BF16)
    asad1_full = nc.dram_tensor("asad1_full", [N, 8], BF16, addr_space="Shared")
    t2_loc = nc.dram_tensor("t2_loc", [NT * 128, TW2], BF16)
    t2_full = nc.dram_tensor("t2_full", [NG * NDEV * GT * 128, TW2], BF16,
                             addr_space="Shared")
    rdscr = nc.dram_tensor("rdscr", [NT, 512], F32)
    pc_loc = nc.dram_tensor("pc_loc", [B, C2 + 1], F32)
    pc_red = nc.dram_tensor("pc_red", [B, C2 + 1], F32, addr_space="Shared")

    RG = [list(range(NDEV))]
    KMAX = max(Ks)

    import os as _os
    DEBUG = bool(int(_os.environ.get("BASS_GAT_DEBUG", "0")))
    if DEBUG:
        dbg_h1 = nc.dram_tensor("dbg_h1", [NT * 128, C1], F32, kind="ExternalOutput")
        dbg_t2 = nc.dram_tensor("dbg_t2", [NG * NDEV * GT * 128, TW2], F32,
                                kind="ExternalOutput")
        dbg_pc = nc.dram_tensor("dbg_pc", [B, C2 + 1], F32, kind="ExternalOutput")

    with tile.TileContext(nc) as tc:
        with (
            tc.tile_pool(name="const", bufs=1) as cp,
            tc.tile_pool(name="small", bufs=5) as sp,
            tc.tile_pool(name="selp", bufs=4) as selp,
            tc.tile_pool(name="selwp", bufs=4) as selwp,
            tc.tile_pool(name="xgp", bufs=2) as xgp,
            tc.tile_pool(name="hgp", bufs=2) as hgp,
            tc.tile_pool(name="dstp", bufs=2) as dstp,
            tc.tile_pool(name="utp", bufs=2) as utp,
            tc.tile_pool(name="t2p", bufs=2) as t2p,
        ):
            # ================= constants =================
            ident = cp.tile([128, 128], F32, tag="ident")
            make_identity(nc, ident[:])
            identb = cp.tile([128, 128], BF16, tag="identb")
            make_identity(nc, identb[:])
            iota_i = cp.tile([128, 128], I32, tag="iota_i")
            nc.gpsimd.iota(iota_i[:], pattern=[[1, 128]], base=0, channel_multiplier=0)
            iotaT = cp.tile([128, 128], F32, tag="iotaT")
            nc.vector.tensor_copy(out=iotaT[:], in_=iota_i[:])
            iota_ci = cp.tile([128, 1], I32, tag="iota_ci")
            nc.gpsimd.iota(iota_ci[:], pattern=[[1, 1]], base=0, channel_multiplier=1)
            iotaC = cp.tile([128, 1], F32, tag="iotaC")
            nc.vector.tensor_copy(out=iotaC[:], in_=iota_ci[:])
            ones1 = cp.tile([1, 128], BF16, tag="ones1")
            nc.vector.memset(ones1[:], 1.0)
            ones1f = cp.tile([1, 128], F32, tag="ones1f")
            nc.vector.memset(ones1f[:], 1.0)

            b1_sb = cp.tile([1, C1], BF16, tag="b1")
            nc.scalar.dma_start(out=b1_sb[:], in_=b1_t[None, :])
            b2bc = cp.tile([128, C2], F32, tag="b2bc")
            nc.scalar.dma_start(out=b2bc[:], in_=b2_t[None, :].to_broadcast([128, C2]))
            fcb_sb = cp.tile([1, 2], F32, tag="fcb")
            nc.scalar.dma_start(out=fcb_sb[:], in_=fcb_t[None, :])
            fcW_sb = cp.tile([128, 4], F32, tag="fcW")
            for c in range(2):
                nc.scalar.dma_start(out=fcW_sb[:, 2 * c:2 * c + 2],
                                  in_=fcW_t[c * 128:(c + 1) * 128, :])

            W1_sb = cp.tile([128, CC1 * HC1], BF16, tag="W1")
            for c in range(CC1):
                nc.scalar.dma_start(out=W1_sb[:, c * HC1:(c + 1) * HC1],
                                  in_=W1_t[c * 128:(c + 1) * 128, :])
            W2c_sb = cp.tile([128, CC2 * W2C], BF16, tag="W2c")
            for c in range(CC2):
                nc.scalar.dma_start(out=W2c_sb[:, c * W2C:(c + 1) * W2C],
                                  in_=W2c_t[c * 128:(c + 1) * 128, :])
            wa1_sb = cp.tile([128, CC1 * 8], BF16, tag="wa1")
            nc.scalar.dma_start(
                out=wa1_sb[:].rearrange("p (c j) -> p c j", j=8),
                in_=wa1_t[:].rearrange("(c p) j -> p c j", p=128))
            batchf_sb = cp.tile([128, NT], F32, tag="batchf")
            nc.scalar.dma_start(out=batchf_sb[:], in_=batchf_t[:, :])
            poolacc = cp.tile([128, C2 + 1], F32, tag="poolacc")
            nc.vector.memset(poolacc[:], 0.0)

            # ============ asad1 = x_loc @ wa1 ============
            with (
                tc.tile_pool(name="prep", bufs=2) as pp,
                tc.tile_pool(name="pshp", bufs=2, space="PSUM") as pshp,
            ):
                for t in range(NT):
                    rows = min(128, NPD - t * 128)
                    xT = pp.tile([128, CC1 * 128], BF16, tag="xT")
                    nc.sync.dma_start(
                        out=xT[:, :CC1 * rows].rearrange("p (c n) -> p c n", c=CC1),
                        in_=xlocT_t[:, t * 128: t * 128 + rows].rearrange(
                            "(c p) n -> p c n", p=128))
                    ps = pshp.tile([128, 8], F32, tag="ps")
                    for c in range(CC1):
                        nc.tensor.matmul(out=ps[:rows, :],
                                         lhsT=xT[:, c * rows:(c + 1) * rows],
                                         rhs=wa1_sb[:, c * 8:(c + 1) * 8],
                                         start=(c == 0), stop=(c == CC1 - 1))
                    as1 = pp.tile([128, 8], BF16, tag="as1")
                    nc.vector.tensor_copy(out=as1[:rows, :], in_=ps[:rows, :])
                    nc.sync.dma_start(out=asad1_loc[t * 128: t * 128 + rows, :],
                                      in_=as1[:rows, :])

            nc.gpsimd.collective_compute(
                "AllGather", mybir.AluOpType.bypass, replica_groups=RG,
                ins=[asad1_loc[:, :]], outs=[asad1_full[:, :]])
            NW = 12
            CH = (N + NW - 1) // NW
            for w in range(NW):
                lo, hi = w * CH, min(N, (w + 1) * CH)
                q = nc.sync
                q.dma_start(out=xa_t[lo:hi, C1IN:C1IN + 8],
                            in_=asad1_full[lo:hi, :])

            # ================= layer 1 sweep =================
            with (
                tc.tile_pool(name="psu", bufs=1, space="PSUM") as psu,
                tc.tile_pool(name="psh", bufs=1, space="PSUM") as psh,
                tc.tile_pool(name="psr", bufs=1, space="PSUM") as psr,
            ):
                for t in range(NT):
                    K = Ks[t]
                    o = offs[t]
                    rows = min(128, NPD - t * 128)

                    idx_sb = sp.tile([128, KMAX], I32, tag="idx")
                    nc.scalar.dma_start(out=idx_sb[:, :K], in_=xidx_t[:, o:o + K])
                    ad_sb = sp.tile([128, 4], BF16, tag="ad")
                    if rows < 128:
                        nc.vector.memset(ad_sb[:], 0.0)
                    nc.scalar.dma_start(out=ad_sb[:rows, :],
                                      in_=asad1_loc[t * 128:t * 128 + rows, 4:8])

                    # ---------- gather x rows (asad embedded) ----------
                    xg = xgp.tile([128, KMAX * XAW], BF16, tag="xg")
                    for k in range(K):
                        nc.gpsimd.indirect_dma_start(
                            out=xg[:, k * XAW:(k + 1) * XAW], out_offset=None,
                            in_=xa_t[:, :],
                            in_offset=IndirectOffsetOnAxis(
                                ap=idx_sb[:, k:k + 1], axis=0))
                    xg3 = xg[:, :K * XAW].rearrange("p (k d) -> p k d", d=XAW)

                    # ---------- selection matrices (batched) ----------
                    selT = selp.tile([128, KMAX * 128], BF16, tag="selT")
                    nc.scalar.dma_start(out=selT[:, :K * 128],
                                        in_=selTN_t[:, o * 128:(o + K) * 128])
                    sel = selp.tile([128, KMAX * 128], BF16, tag="sel")
                    nc.scalar.dma_start(out=sel[:, :K * 128],
                                        in_=selN_t[:, o * 128:(o + K) * 128])

                    # ---------- e = lrelu(as+ad); ex = exp ----------
                    ade_ps = psh.tile([128, 4 * KMAX], F32, tag="hold")
                    for k in range(K):
                        nc.tensor.matmul(out=ade_ps[:, 4 * k:4 * k + 4],
                                         lhsT=selT[:, k * 128:(k + 1) * 128],
                                         rhs=ad_sb[:],
                                         start=(k == 0), stop=(k == K - 1))
                    asg = sp.tile([128, 4 * KMAX], F32, tag="asg")
                    nc.vector.tensor_copy(out=asg[:, :4 * K].rearrange(
                        "p (k s) -> p k s", s=4),
                        in_=xg3[:, :, C1IN:C1IN + 4])
                    z = sp.tile([128, 4 * KMAX], F32, tag="z")
                    nc.vector.tensor_tensor(out=z[:, :4 * K], in0=ade_ps[:, :4 * K],
                                            in1=asg[:, :4 * K],
                                            op=mybir.AluOpType.add)
                    zs = sp.tile([128, 4 * KMAX], F32, tag="zs")
                    nc.vector.tensor_scalar_mul(zs[:, :4 * K], z[:, :4 * K], NEG)
                    nc.vector.tensor_tensor(out=z[:, :4 * K], in0=z[:, :4 * K],
                                            in1=zs[:, :4 * K],
                                            op=mybir.AluOpType.max)
                    exf = sp.tile([128, 4 * KMAX], F32, tag="exf")
                    nc.scalar.activation(out=exf[:, :4 * K], in_=z[:, :4 * K],
                                         func=mybir.ActivationFunctionType.Exp)
                    ex = sp.tile([128, 4 * KMAX], BF16, tag="ex")
                    nc.vector.tensor_copy(out=ex[:, :4 * K], in_=exf[:, :4 * K])

                    # ---------- denT[h, d] = sum_e ex[e,h] sel[e,d] ----------
                    denT_ps = psh.tile([4, 128], F32, tag="hold")
                    for k in range(K):
                        nc.tensor.matmul(out=denT_ps[:],
                                         lhsT=ex[:, 4 * k:4 * k + 4],
                                         rhs=sel[:, k * 128:(k + 1) * 128],
                                         start=(k == 0), stop=(k == K - 1))

                    # ---------- aggregate ut[f,(h,d)] += x ex ----------
                    ut_ps = psu.tile([128, CC1 * 512], F32, tag="ut")
                    for k in range(K):
                        selw = selwp.tile([128, 512], BF16, tag="selw")
                        for h in range(H):
                            nc.vector.tensor_scalar_mul(
                                selw[:, h * 128:(h + 1) * 128],
                                sel[:, k * 128:(k + 1) * 128],
                                exf[:, 4 * k + h:4 * k + h + 1])
                        for c in range(CC1):
                            nc.tensor.matmul(
                                out=ut_ps[:, c * 512:(c + 1) * 512],
                                lhsT=xg[:, k * XAW + c * 128: k * XAW + (c + 1) * 128],
                                rhs=selw[:],
                                start=(k == 0), stop=(k == K - 1))

                    # ---------- rdenT broadcast + divide ----------
                    rdT = sp.tile([4, 128], F32, tag="rdT")
                    nc.vector.tensor_scalar(out=rdT[:], in0=denT_ps[:],
                                            scalar1=4.0, scalar2=1e-16,
                                            op0=mybir.AluOpType.mult,
                                            op1=mybir.AluOpType.max)
                    nc.vector.reciprocal(out=rdT[:], in_=rdT[:])
                    nc.sync.dma_start(
                        out=rdscr[t:t + 1, :].rearrange("a (p f) -> (a p) f", p=4),
                        in_=rdT[:])
                    rdb = dstp.tile([128, 512], F32, tag="rdb")
                    nc.sync.dma_start(out=rdb[:],
                                      in_=rdscr[t:t + 1, :].to_broadcast([128, 512]))
                    ut_sb = utp.tile([128, CC1 * 512], BF16, tag="ut")
                    for c in range(CC1):
                        nc.vector.tensor_tensor(out=ut_sb[:, c * 512:(c + 1) * 512],
                                                in0=ut_ps[:, c * 512:(c + 1) * 512],
                                                in1=rdb[:],
                                                op=mybir.AluOpType.mult)

                    # ---------- project + bias ----------
                    out_ps = psh.tile([128, C1], F32, tag="hold")
                    first = True
                    for c in range(CC1):
                        for h in range(H):
                            nc.tensor.matmul(
                                out=out_ps[:],
                                lhsT=ut_sb[:, c * 512 + h * 128: c * 512 + (h + 1) * 128],
                                rhs=W1_sb[:, c * HC1 + h * C1: c * HC1 + (h + 1) * C1],
                                start=first, stop=False)
                            first = False
                    nc.tensor.matmul(out=out_ps[:], lhsT=ones1[:], rhs=b1_sb[:],
                                     start=False, stop=True)
                    h1_sb = t2p.tile([128, C1], BF16, tag="h1")
                    nc.vector.tensor_copy(out=h1_sb[:], in_=out_ps[:])
                    if DEBUG:
                        h1f = sp.tile([128, C1], F32, tag="h1f")
                        nc.vector.tensor_copy(out=h1f[:], in_=out_ps[:])
                        nc.sync.dma_start(out=dbg_h1[t * 128:(t + 1) * 128, :],
                                          in_=h1f[:])

                    # ---------- xw2 = h1 @ [W2 | wa2] ----------
                    h1T = t2p.tile([128, C1], BF16, tag="h1T")
                    for c in range(CC2):
                        tp = psr.tile([128, 128], BF16, tag="rot")
                        nc.tensor.transpose(out=tp[:],
                                            in_=h1_sb[:, c * 128:(c + 1) * 128],
                                            identity=identb[:])
                        nc.vector.tensor_copy(out=h1T[:, c * 128:(c + 1) * 128],
                                              in_=tp[:])
                    xw2_sb = t2p.tile([128, TW2], BF16, tag="xw2")
                    for (lo, w) in ((0, 512), (512, 512), (1024, 8)):
                        xp = psr.tile([128, 512], F32, tag="rot")
                        for c in range(CC2):
                            nc.tensor.matmul(
                                out=xp[:, :w],
                                lhsT=h1T[:, c * 128:(c + 1) * 128],
                                rhs=W2c_sb[:, c * W2C + lo: c * W2C + lo + w],
                                start=(c == 0), stop=(c == CC2 - 1))
                        nc.vector.tensor_copy(out=xw2_sb[:, lo:lo + w], in_=xp[:, :w])
                    nc.sync.dma_start(out=t2_loc[t * 128:(t + 1) * 128, :],
                                      in_=xw2_sb[:])

                    if t % GT == GT - 1:
                        g = t // GT
                        nc.gpsimd.collective_compute(
                            "AllGather", mybir.AluOpType.bypass, replica_groups=RG,
                            ins=[t2_loc[g * GT * 128:(g + 1) * GT * 128, :]],
                            outs=[t2_full[g * NDEV * GT * 128:
                                          (g + 1) * NDEV * GT * 128, :]])

            if DEBUG:
                t2f = sp.tile([128, TW2], F32, tag="t2f")
                for i in range(NG * NDEV * GT):
                    t2b = sp.tile([128, TW2], BF16, tag="t2b")
                    nc.sync.dma_start(out=t2b[:],
                                      in_=t2_full[i * 128:(i + 1) * 128, :])
                    nc.vector.tensor_copy(out=t2f[:], in_=t2b[:])
                    nc.sync.dma_start(out=dbg_t2[i * 128:(i + 1) * 128, :],
                                      in_=t2f[:])

            # ================= layer 2 sweep + pooling =================
            with (
                tc.tile_pool(name="psu2", bufs=1, space="PSUM") as psu2,
                tc.tile_pool(name="psh2", bufs=2, space="PSUM") as psh2,
            ):
                for t in range(NT):
                    K = Ks[t]
                    o = offs[t]
                    rows = min(128, NPD - t * 128)

                    idx_sb = sp.tile([128, KMAX], I32, tag="idx")
                    nc.scalar.dma_start(out=idx_sb[:, :K], in_=x2idx_t[:, o:o + K])
                    ad_sb = sp.tile([128, 4], BF16, tag="ad")
                    if rows < 128:
                        nc.vector.memset(ad_sb[:], 0.0)
                    nc.scalar.dma_start(out=ad_sb[:rows, :],
                                      in_=t2_loc[t * 128:t * 128 + rows,
                                                 HC2 + 4:HC2 + 8])

                    hg = hgp.tile([128, KMAX * TW2], BF16, tag="hg")
                    for k in range(K):
                        nc.gpsimd.indirect_dma_start(
                            out=hg[:, k * TW2:(k + 1) * TW2], out_offset=None,
                            in_=t2_full[:, :],
                            in_offset=IndirectOffsetOnAxis(
                                ap=idx_sb[:, k:k + 1], axis=0))
                    hg3 = hg[:, :K * TW2].rearrange("p (k d) -> p k d", d=TW2)

                    selT = selp.tile([128, KMAX * 128], BF16, tag="selT")
                    nc.scalar.dma_start(out=selT[:, :K * 128],
                                        in_=selTN_t[:, o * 128:(o + K) * 128])
                    sel = selp.tile([128, KMAX * 128], BF16, tag="sel")
                    nc.scalar.dma_start(out=sel[:, :K * 128],
                                        in_=selN_t[:, o * 128:(o + K) * 128])

                    ade_ps = psh2.tile([128, 4 * KMAX], F32, tag="hold")
                    for k in range(K):
                        nc.tensor.matmul(out=ade_ps[:, 4 * k:4 * k + 4],
                                         lhsT=selT[:, k * 128:(k + 1) * 128],
                                         rhs=ad_sb[:],
                                         start=(k == 0), stop=(k == K - 1))
                    asg = sp.tile([128, 4 * KMAX], F32, tag="asg")
                    nc.vector.tensor_copy(out=asg[:, :4 * K].rearrange(
                        "p (k s) -> p k s", s=4),
                        in_=hg3[:, :, HC2:HC2 + 4])
                    z = sp.tile([128, 4 * KMAX], F32, tag="z")
                    nc.vector.tensor_tensor(out=z[:, :4 * K], in0=ade_ps[:, :4 * K],
                                            in1=asg[:, :4 * K],
                                            op=mybir.AluOpType.add)
                    zs = sp.tile([128, 4 * KMAX], F32, tag="zs")
                    nc.vector.tensor_scalar_mul(zs[:, :4 * K], z[:, :4 * K], NEG)
                    nc.vector.tensor_tensor(out=z[:, :4 * K], in0=z[:, :4 * K],
                                            in1=zs[:, :4 * K],
                                            op=mybir.AluOpType.max)
                    exf = sp.tile([128, 4 * KMAX], F32, tag="exf")
                    nc.scalar.activation(out=exf[:, :4 * K], in_=z[:, :4 * K],
                                         func=mybir.ActivationFunctionType.Exp)
                    ex = sp.tile([128, 4 * KMAX], BF16, tag="ex")
                    nc.vector.tensor_copy(out=ex[:, :4 * K], in_=exf[:, :4 * K])

                    # agg[d, (h,c)] += ex * xw2 ; den[d, h] += ex
                    agg_ps = psu2.tile([128, 4 * 512], F32, tag="ut2")
                    den_ps = psh2.tile([128, 4], F32, tag="hold")
                    for k in range(K):
                        selh = selwp.tile([128, 512], BF16, tag="selh")
                        for h in range(H):
                            nc.vector.tensor_scalar_mul(
                                selh[:, h * 128:(h + 1) * 128],
                                sel[:, k * 128:(k + 1) * 128],
                                exf[:, 4 * k + h:4 * k + h + 1])
                        for h in range(H):
                            nc.tensor.matmul(
                                out=agg_ps[:, h * 512:h * 512 + C2],
                                lhsT=selh[:, h * 128:(h + 1) * 128],
                                rhs=hg[:, k * TW2 + h * C2: k * TW2 + (h + 1) * C2],
                                start=(k == 0), stop=(k == K - 1))
                        nc.tensor.matmul(out=den_ps[:],
                                         lhsT=sel[:, k * 128:(k + 1) * 128],
                                         rhs=ex[:, 4 * k:4 * k + 4],
                                         start=(k == 0), stop=(k == K - 1))

                    rd2 = sp.tile([128, 4], F32, tag="rd2")
                    nc.vector.tensor_scalar(out=rd2[:], in0=den_ps[:],
                                            scalar1=4.0, scalar2=1e-16,
                                            op0=mybir.AluOpType.mult,
                                            op1=mybir.AluOpType.max)
                    nc.vector.reciprocal(out=rd2[:], in_=rd2[:])
                    h2acc = sp.tile([128, C2], F32, tag="h2acc")
                    nc.vector.tensor_scalar_mul(h2acc[:], agg_ps[:, 0:C2],
                                                rd2[:, 0:1])
                    for h in range(1, H):
                        nc.vector.scalar_tensor_tensor(
                            out=h2acc[:], in0=agg_ps[:, h * 512:h * 512 + C2],
                            scalar=rd2[:, h:h + 1], in1=h2acc[:],
                            op0=mybir.AluOpType.mult, op1=mybir.AluOpType.add)
                    h2p = t2p.tile([128, C2 + 1], BF16, tag="h2p")
                    nc.vector.tensor_tensor(out=h2p[:, :C2], in0=h2acc[:],
                                            in1=b2bc[:], op=mybir.AluOpType.add)
                    nc.vector.memset(h2p[:, C2:C2 + 1], 1.0)

                    selB = sp.tile([128, 128], BF16, tag="selB")
                    nc.vector.tensor_tensor(
                        out=selB[:],
                        in0=batchf_sb[:, t:t + 1].to_broadcast([128, 128]),
                        in1=iotaT[:], op=mybir.AluOpType.is_equal)
                    pc_ps = psh2.tile([128, C2 + 1], F32, tag="hold")
                    nc.tensor.matmul(out=pc_ps[:], lhsT=selB[:], rhs=h2p[:],
                                     start=True, stop=True)
                    nc.vector.tensor_tensor(out=poolacc[:], in0=poolacc[:],
                                            in1=pc_ps[:], op=mybir.AluOpType.add)

                # ================= pool reduce + FC =================
                nc.sync.dma_start(out=pc_loc[:, :], in_=poolacc[:])
                nc.gpsimd.collective_compute(
                    "AllReduce", mybir.AluOpType.add, replica_groups=RG,
                    ins=[pc_loc[:, :]], outs=[pc_red[:, :]])
                pc_sb = sp.tile([128, C2 + 1], F32, tag="pc")
                nc.sync.dma_start(out=pc_sb[:], in_=pc_red[:, :])
                if DEBUG:
                    nc.sync.dma_start(out=dbg_pc[:, :], in_=pc_red[:, :])
                cnt = sp.tile([128, 1], F32, tag="cnt")
                nc.vector.tensor_scalar_max(cnt[:], pc_sb[:, C2:C2 + 1], 1.0)
                nc.vector.reciprocal(out=cnt[:], in_=cnt[:])
                g_sb = sp.tile([128, C2], F32, tag="g")
                nc.vector.tensor_scalar_mul(g_sb[:], pc_sb[:, :C2], cnt[:, :1])

                y_ps = psh2.tile([128, 2], F32, tag="hold")
                for c in range(2):
                    tp = psu2.tile([128, 128], F32, tag="ut2")
                    nc.tensor.transpose(out=tp[:], in_=g_sb[:, c * 128:(c + 1) * 128],
                                        identity=ident[:])
                    gT = sp.tile([128, 128], F32, tag="gT")
                    nc.vector.tensor_copy(out=gT[:], in_=tp[:])
                    nc.tensor.matmul(out=y_ps[:], lhsT=gT[:],
                                     rhs=fcW_sb[:, 2 * c:2 * c + 2],
                                     start=(c == 0), stop=False)
                nc.tensor.matmul(out=y_ps[:], lhsT=ones1f[:], rhs=fcb_sb[:],
                                 start=False, stop=True)
                y_sb = sp.tile([128, 2], F32, tag="y")
                nc.vector.tensor_copy(out=y_sb[:], in_=y_ps[:])
                nc.sync.dma_start(out=y_t[:, :], in_=y_sb[:])

    nc.compile()
    return nc


_CACHE = {}


def kernel(**inputs):
    x = np.ascontiguousarray(np.asarray(inputs["x"], dtype=np.float32))
    edge_index = np.asarray(inputs["edge_index"])
    batch = np.asarray(inputs["batch"])
    W1 = np.asarray(inputs["W1"], dtype=np.float32)
    W2 = np.asarray(inputs["W2"], dtype=np.float32)
    a_src1 = np.asarray(inputs["a_src1"], dtype=np.float32)
    a_dst1 = np.asarray(inputs["a_dst1"], dtype=np.float32)
    a_src2 = np.asarray(inputs["a_src2"], dtype=np.float32)
    a_dst2 = np.asarray(inputs["a_dst2"], dtype=np.float32)
    b1 = np.asarray(inputs["b1"], dtype=np.float32)
    b2 = np.asarray(inputs["b2"], dtype=np.float32)
    fcW = np.ascontiguousarray(np.asarray(inputs["fcW"], dtype=np.float32))
    fcb = np.asarray(inputs["fcb"], dtype=np.float32)

    Ks, offs, SK, xidx, x2idx, selN, selTN, batchf = _host_prep(edge_index, batch)

    key = (tuple(Ks),)
    if key not in _CACHE:
        _CACHE[key] = _build(Ks, offs, SK)
    nc = _CACHE[key]

    # weight-only prep: wa = W^T a per head (folded attention projections)
    wa1 = np.zeros((C1IN, 8), dtype=np.float32)
    wa2 = np.zeros((C2IN, 8), dtype=np.float32)
    for h in range(H):
        wa1[:, h] = W1[:, h * C1:(h + 1) * C1] @ a_src1[h]
        wa1[:, 4 + h] = W1[:, h * C1:(h + 1) * C1] @ a_dst1[h]
        wa2[:, h] = W2[:, h * C2:(h + 1) * C2] @ a_src2[h]
        wa2[:, 4 + h] = W2[:, h * C2:(h + 1) * C2] @ a_dst2[h]
    W2c = np.concatenate([W2, wa2], axis=1)

    xa = np.zeros((N, XAW), dtype=BFNP)
    xa[:, :C1IN] = x.astype(BFNP)

    in_maps = []
    for d in range(NDEV):
        xloc = x[d * NPD:(d + 1) * NPD]
        in_maps.append({
            "xa": xa,
            "xlocT": np.ascontiguousarray(xloc.T).astype(BFNP),
            "W1": W1.astype(BFNP), "W2c": W2c.astype(BFNP),
            "wa1": wa1.astype(BFNP), "b1": b1.astype(BFNP), "b2": b2,
            "fcW": fcW, "fcb": fcb,
            "xidx": xidx[d], "x2idx": x2idx[d], "selN": selN[d],
            "selTN": selTN[d], "batchf": batchf[d],
        })

    import os as _os
    trace = bool(int(_os.environ.get("BASS_GAT_TRACE", "0")))
    kwargs = {}
    if trace:
        _setup_ntff_hook()
        kwargs = dict(trace=True, trace_cores=[0])
    res = run_bass_kernel_spmd(nc, in_maps, core_ids=list(range(NDEV)), **kwargs)
    if trace:
        kernel.last_exec_ns = res.exec_time_ns
        kernel.last_trace = res.instructions_and_trace
        if res.exec_time_ns is not None:
            print(f"HW exec time: {res.exec_time_ns} ns")
    if bool(int(_os.environ.get("BASS_GAT_DEBUG", "0"))):
        kernel.debug_results = res.results
    return res.results[0]["y"]


def _setup_ntff_hook():
    """The image's antenv lacks axon_hooks; synthesize it and register the
    ctypes NTFF profiling hook so trace=True works."""
    import types
    import antenv
    if hasattr(antenv, "axon_hooks"):
        return
    mod = types.ModuleType("antenv.axon_hooks")
    state = {"hook": None}
    mod.set_axon_ntff_profile_hook = lambda h: state.__setitem__("hook", h)
    mod.get_axon_ntff_profile_hook = lambda: state["hook"]
    sys.modules["antenv.axon_hooks"] = mod
    antenv.axon_hooks = mod
    try:
        from trn_agent_boot.trn_boot import _ntff_profile_via_ctypes
        hook = _ntff_profile_via_ctypes("/opt/axon/libaxon_pjrt.so")
        mod.set_axon_ntff_profile_hook(hook)
    except Exception as e:
        print("ntff hook setup failed:", e)


# revision 28
# speedup vs baseline: 1.2476x; 1.1597x over previous
import sys
import numpy as np

sys.path.insert(0, "/opt/trn_rl_repo")
sys.path.insert(0, "/opt/trn_rl_repo/concourse")

import ml_dtypes
import concourse.bass as bass
import concourse.bacc as bacc
import concourse.mybir as mybir
import concourse.tile as tile
from concourse.bass import IndirectOffsetOnAxis
from concourse.bass_utils import run_bass_kernel_spmd
from concourse.masks import make_identity

F32 = mybir.dt.float32
BF16 = mybir.dt.bfloat16
I32 = mybir.dt.int32
BFNP = ml_dtypes.bfloat16

N = 20000
E = 160000
B = 128
NDEV = 8
NPD = N // NDEV          # 2500 nodes per device
NT = (NPD + 127) // 128  # 20 dst tiles per device
GT = 2                   # tiles per allgather group
NG = NT // GT            # 5 groups
H = 4
C1IN, C1 = 768, 512
C2IN, C2 = 512, 256
CC1 = C1IN // 128        # 6
CC2 = C2IN // 128        # 4
HC1 = H * C1             # 2048
HC2 = H * C2             # 1024
XAW = 800                # [x 768 | as1 4 | ad1 4 | pad] bf16 row (1600B)
TW2 = 1056               # [xw2 1024 | as2 4 | ad2 4 | pad] bf16 row (2112B)
W2C = HC2 + 8            # 1032 cols of [W2 | wa2]
NEG = 0.2


def _host_prep(edge_index, batch):
    """Integer-only preprocessing: edge partitioning, sorting, chunk layout."""
    src = np.concatenate([edge_index[0], np.arange(N, dtype=np.int64)]).astype(np.int64)
    dst = np.concatenate([edge_index[1], np.arange(N, dtype=np.int64)]).astype(np.int64)
    order = np.argsort(dst, kind="stable")
    src, dst = src[order], dst[order]

    dev = dst // NPD
    tloc = (dst % NPD) // 128
    cnt = np.zeros((NDEV, NT), dtype=np.int64)
    for d in range(NDEV):
        m = dev == d
        cnt[d] = np.bincount(tloc[m], minlength=NT)
    Ks = [max(1, int(np.ceil(cnt[:, t].max() / 128.0))) for t in range(NT)]
    SK = sum(Ks)
    offs = np.cumsum([0] + Ks)

    # t2_full row index for source node s (grouped allgather layout)
    s_dev = src // NPD
    s_loc = src % NPD
    s_tl = s_loc // 128
    s_r = s_loc % 128
    t2row = (s_tl // GT) * (NDEV * GT * 128) + s_dev * (GT * 128) + (s_tl % GT) * 128 + s_r

    xidx = np.zeros((NDEV, 128, SK), dtype=np.int32)   # into xa rows
    x2idx = np.zeros((NDEV, 128, SK), dtype=np.int32)  # into t2_full rows
    dstf = np.full((NDEV, 128, SK), -1.0, dtype=np.float32)
    dstfR = np.full((NDEV, SK, 128), -1.0, dtype=np.float32)

    for d in range(NDEV):
        m = dev == d
        s_d, t_d, dl_d, r2_d = src[m], tloc[m], (dst[m] % NPD) % 128, t2row[m]
        for t in range(NT):
            mt = t_d == t
            s_t = s_d[mt]
            dl_t = dl_d[mt]
            r2_t = r2_d[mt]
            o = offs[t]
            j = np.arange(len(s_t))
            xidx[d, j % 128, o + j // 128] = s_t
            x2idx[d, j % 128, o + j // 128] = r2_t
            dstf[d, j % 128, o + j // 128] = dl_t.astype(np.float32)
            dstfR[d, o + j // 128, j % 128] = dl_t.astype(np.float32)

    iota = np.arange(128, dtype=np.float32)
    selN = (dstf[:, :, :, None] == iota).astype(BFNP).reshape(NDEV, 128, SK * 128)
    selTN = (iota[None, :, None, None] == dstfR[:, None, :, :]).astype(BFNP)
    selTN = selTN.reshape(NDEV, 128, SK * 128)

    batchf = np.full((NDEV, 128, NT), -1.0, dtype=np.float32)
    b_np = np.asarray(batch).astype(np.int64)
    for d in range(NDEV):
        for t in range(NT):
            rows = min(128, NPD - t * 128)
            g = b_np[d * NPD + t * 128: d * NPD + t * 128 + rows]
            batchf[d, :rows, t] = g.astype(np.float32)

    return Ks, offs, SK, xidx, x2idx, selN, selTN, batchf


def _build(Ks, offs, SK):
    """Emit the Bass program (identical for all 8 cores)."""
    nc = bacc.Bacc("TRN2", target_bir_lowering=False, debug=False, num_devices=NDEV)

    # ---- I/O ----
    xa_t = nc.dram_tensor("xa", [N, XAW], BF16, kind="ExternalInput")
    xlocT_t = nc.dram_tensor("xlocT", [C1IN, NPD], BF16, kind="ExternalInput")
    W1_t = nc.dram_tensor("W1", [C1IN, HC1], BF16, kind="ExternalInput")
    W2c_t = nc.dram_tensor("W2c", [C2IN, W2C], BF16, kind="ExternalInput")
    wa1_t = nc.dram_tensor("wa1", [C1IN, 8], BF16, kind="ExternalInput")
    b1_t = nc.dram_tensor("b1", [C1], BF16, kind="ExternalInput")
    b2_t = nc.dram_tensor("b2", [C2], F32, kind="ExternalInput")
    fcW_t = nc.dram_tensor("fcW", [C2, 2], F32, kind="ExternalInput")
    fcb_t = nc.dram_tensor("fcb", [2], F32, kind="ExternalInput")
    xidx_t = nc.dram_tensor("xidx", [128, SK], I32, kind="ExternalInput")
    x2idx_t = nc.dram_tensor("x2idx", [128, SK], I32, kind="ExternalInput")
    selN_t = nc.dram_tensor("selN", [128, SK * 128], BF16, kind="ExternalInput")
    selTN_t = nc.dram_tensor("selTN", [128, SK * 128], BF16, kind="ExternalInput")
    batchf_t = nc.dram_tensor("batchf", [128, NT], F32, kind="ExternalInput")
    y_t = nc.dram_tensor("y", [B, 2], F32, kind="ExternalOutput")

    # ---- internal DRAM ----
    asad1_loc = nc.dram_tensor("asad1_loc", [NPD, 8], BF16)
    asad1_full = nc.dram_tensor("asad1_full", [N, 8], BF16, addr_space="Shared")
    t2_loc = nc.dram_tensor("t2_loc", [NT * 128, TW2], BF16)
    t2_full = nc.dram_tensor("t2_full", [NG * NDEV * GT * 128, TW2], BF16,
                             addr_space="Shared")
    rdscr = nc.dram_tensor("rdscr", [NT, 512], F32)
    pc_loc = nc.dram_tensor("pc_loc", [B, C2 + 1], F32)
    pc_red = nc.dram_tensor("pc_red", [B, C2 + 1], F32, addr_space="Shared")

    RG = [list(range(NDEV))]
    KMAX = max(Ks)

    import os as _os
    DEBUG = bool(int(_os.environ.get("BASS_GAT_DEBUG", "0")))
    if DEBUG:
        dbg_h1 = nc.dram_tensor("dbg_h1", [NT * 128, C1], F32, kind="ExternalOutput")
        dbg_t2 = nc.dram_tensor("dbg_t2", [NG * NDEV * GT * 128, TW2], F32,
                                kind="ExternalOutput")
        dbg_pc = nc.dram_tensor("dbg_pc", [B, C2 + 1], F32, kind="ExternalOutput")

    with tile.TileContext(nc) as tc:
        with (
            tc.tile_pool(name="const", bufs=1) as cp,
            tc.tile_pool(name="small", bufs=5) as sp,
            tc.tile_pool(name="selp", bufs=4) as selp,
            tc.tile_pool(name="selwp", bufs=4) as selwp,
            tc.tile_pool(name="xgp", bufs=2) as xgp,
            tc.tile_pool(name="hgp", bufs=2) as hgp,
            tc.tile_pool(name="dstp", bufs=2) as dstp,
            tc.tile_pool(name="utp", bufs=2) as utp,
            tc.tile_pool(name="t2p", bufs=2) as t2p,
        ):
            # ================= constants =================
            ident = cp.tile([128, 128], F32, tag="ident")
            make_identity(nc, ident[:])
            identb = cp.tile([128, 128], BF16, tag="identb")
            make_identity(nc, identb[:])
            iota_i = cp.tile([128, 128], I32, tag="iota_i")
            nc.gpsimd.iota(iota_i[:], pattern=[[1, 128]], base=0, channel_multiplier=0)
            iotaT = cp.tile([128, 128], F32, tag="iotaT")
            nc.vector.tensor_copy(out=iotaT[:], in_=iota_i[:])
            iota_ci = cp.tile([128, 1], I32, tag="iota_ci")
            nc.gpsimd.iota(iota_ci[:], pattern=[[1, 1]], base=0, channel_multiplier=1)
            iotaC = cp.tile([128, 1], F32, tag="iotaC")
            nc.vector.tensor_copy(out=iotaC[:], in_=iota_ci[:])
            ones1 = cp.tile([1, 128], BF16, tag="ones1")
            nc.vector.memset(ones1[:], 1.0)
            ones1f = cp.tile([1, 128], F32, tag="ones1f")
            nc.vector.memset(ones1f[:], 1.0)

            b1_sb = cp.tile([1, C1], BF16, tag="b1")
            nc.scalar.dma_start(out=b1_sb[:], in_=b1_t[None, :])
            b2bc = cp.tile([128, C2], F32, tag="b2bc")
            nc.scalar.dma_start(out=b2bc[:], in_=b2_t[None, :].to_broadcast([128, C2]))
            fcb_sb = cp.tile([1, 2], F32, tag="fcb")
            nc.scalar.dma_start(out=fcb_sb[:], in_=fcb_t[None, :])
            fcW_sb = cp.tile([128, 4], F32, tag="fcW")
            for c in range(2):
                nc.scalar.dma_start(out=fcW_sb[:, 2 * c:2 * c + 2],
                                  in_=fcW_t[c * 128:(c + 1) * 128, :])

            W1_sb = cp.tile([128, CC1 * HC1], BF16, tag="W1")
            for c in range(CC1):
                nc.scalar.dma_start(out=W1_sb[:, c * HC1:(c + 1) * HC1],
                                  in_=W1_t[c * 128:(c + 1) * 128, :])
            W2c_sb = cp.tile([128, CC2 * W2C], BF16, tag="W2c")
            for c in range(CC2):
                nc.scalar.dma_start(out=W2c_sb[:, c * W2C:(c + 1) * W2C],
                                  in_=W2c_t[c * 128:(c + 1) * 128, :])
            wa1_sb = cp.tile([128, CC1 * 8], BF16, tag="wa1")
            nc.scalar.dma_start(
                out=wa1_sb[:].rearrange("p (c j) -> p c j", j=8),
                in_=wa1_t[:].rearrange("(c p) j -> p c j", p=128))
            batchf_sb = cp.tile([128, NT], F32, tag="batchf")
            nc.scalar.dma_start(out=batchf_sb[:], in_=batchf_t[:, :])
            poolacc = cp.tile([128, C2 + 1], F32, tag="poolacc")
            nc.vector.memset(poolacc[:], 0.0)

            # ============ asad1 = x_loc @ wa1 ============
            with (
                tc.tile_pool(name="prep", bufs=2) as pp,
                tc.tile_pool(name="pshp", bufs=2, space="PSUM") as pshp,
            ):
                for t in range(NT):
                    rows = min(128, NPD - t * 128)
                    xT = pp.tile([128, CC1 * 128], BF16, tag="xT")
                    nc.sync.dma_start(
                        out=xT[:, :CC1 * rows].rearrange("p (c n) -> p c n", c=CC1),
                        in_=xlocT_t[:, t * 128: t * 128 + rows].rearrange(
                            "(c p) n -> p c n", p=128))
                    ps = pshp.tile([128, 8], F32, tag="ps")
                    for c in range(CC1):
                        nc.tensor.matmul(out=ps[:rows, :],
                                         lhsT=xT[:, c * rows:(c + 1) * rows],
                                         rhs=wa1_sb[:, c * 8:(c + 1) * 8],
                                         start=(c == 0), stop=(c == CC1 - 1))
                    as1 = pp.tile([128, 8], BF16, tag="as1")
                    nc.vector.tensor_copy(out=as1[:rows, :], in_=ps[:rows, :])
                    nc.sync.dma_start(out=asad1_loc[t * 128: t * 128 + rows, :],
                                      in_=as1[:rows, :])

            nc.gpsimd.collective_compute(
                "AllGather", mybir.AluOpType.bypass, replica_groups=RG,
                ins=[asad1_loc[:, :]], outs=[asad1_full[:, :]])
            nc.sync.dma_start(out=xa_t[:, C1IN:C1IN + 8], in_=asad1_full[:, :])

            # ================= layer 1 sweep =================
            with (
                tc.tile_pool(name="psu", bufs=1, space="PSUM") as psu,
                tc.tile_pool(name="psh", bufs=1, space="PSUM") as psh,
                tc.tile_pool(name="psr", bufs=1, space="PSUM") as psr,
            ):
                for t in range(NT):
                    K = Ks[t]
                    o = offs[t]
                    rows = min(128, NPD - t * 128)

                    idx_sb = sp.tile([128, KMAX], I32, tag="idx")
                    nc.scalar.dma_start(out=idx_sb[:, :K], in_=xidx_t[:, o:o + K])
                    ad_sb = sp.tile([128, 4], BF16, tag="ad")
                    if rows < 128:
                        nc.vector.memset(ad_sb[:], 0.0)
                    nc.scalar.dma_start(out=ad_sb[:rows, :],
                                      in_=asad1_loc[t * 128:t * 128 + rows, 4:8])

                    # ---------- gather x rows (asad embedded) ----------
                    xg = xgp.tile([128, KMAX * XAW], BF16, tag="xg")
                    for k in range(K):
                        nc.gpsimd.indirect_dma_start(
                            out=xg[:, k * XAW:(k + 1) * XAW], out_offset=None,
                            in_=xa_t[:, :],
                            in_offset=IndirectOffsetOnAxis(
                                ap=idx_sb[:, k:k + 1], axis=0))
                    xg3 = xg[:, :K * XAW].rearrange("p (k d) -> p k d", d=XAW)

                    # ---------- selection matrices (batched) ----------
                    selT = selp.tile([128, KMAX * 128], BF16, tag="selT")
                    nc.scalar.dma_start(out=selT[:, :K * 128],
                                        in_=selTN_t[:, o * 128:(o + K) * 128])
                    sel = selp.tile([128, KMAX * 128], BF16, tag="sel")
                    nc.scalar.dma_start(out=sel[:, :K * 128],
                                        in_=selN_t[:, o * 128:(o + K) * 128])

                    # ---------- e = lrelu(as+ad); ex = exp ----------
                    ade_ps = psh.tile([128, 4 * KMAX], F32, tag="hold")
                    for k in range(K):
                        nc.tensor.matmul(out=ade_ps[:, 4 * k:4 * k + 4],
                                         lhsT=selT[:, k * 128:(k + 1) * 128],
                                         rhs=ad_sb[:],
                                         start=(k == 0), stop=(k == K - 1))
                    asg = sp.tile([128, 4 * KMAX], F32, tag="asg")
                    nc.vector.tensor_copy(out=asg[:, :4 * K].rearrange(
                        "p (k s) -> p k s", s=4),
                        in_=xg3[:, :, C1IN:C1IN + 4])
                    z = sp.tile([128, 4 * KMAX], F32, tag="z")
                    nc.vector.tensor_tensor(out=z[:, :4 * K], in0=ade_ps[:, :4 * K],
                                            in1=asg[:, :4 * K],
                                            op=mybir.AluOpType.add)
                    zs = sp.tile([128, 4 * KMAX], F32, tag="zs")
                    nc.vector.tensor_scalar_mul(zs[:, :4 * K], z[:, :4 * K], NEG)
                    nc.vector.tensor_tensor(out=z[:, :4 * K], in0=z[:, :4 * K],
                                            in1=zs[:, :4 * K],
                                            op=mybir.AluOpType.max)
                    exf = sp.tile([128, 4 * KMAX], F32, tag="exf")
                    nc.scalar.activation(out=exf[:, :4 * K], in_=z[:, :4 * K],
                                         func=mybir.ActivationFunctionType.Exp)
                    ex = sp.tile([128, 4 * KMAX], BF16, tag="ex")
                    nc.vector.tensor_copy(out=ex[:, :4 * K], in_=exf[:, :4 * K])

                    # ---------- denT[h, d] = sum_e ex[e,h] sel[e,d] ----------
                    denT_ps = psh.tile([4, 128], F32, tag="hold")
                    for k in range(K):
                        nc.tensor.matmul(out=denT_ps[:],
                                         lhsT=ex[:, 4 * k:4 * k + 4],
                                         rhs=sel[:, k * 128:(k + 1) * 128],
                                         start=(k == 0), stop=(k == K - 1))

                    # ---------- aggregate ut[f,(h,d)] += x ex ----------
                    ut_ps = psu.tile([128, CC1 * 512], F32, tag="ut")
                    for k in range(K):
                        selw = selwp.tile([128, 512], BF16, tag="selw")
                        for h in range(H):
                            nc.vector.tensor_scalar_mul(
                                selw[:, h * 128:(h + 1) * 128],
                                sel[:, k * 128:(k + 1) * 128],
                                exf[:, 4 * k + h:4 * k + h + 1])
                        for c in range(CC1):
                            nc.tensor.matmul(
                                out=ut_ps[:, c * 512:(c + 1) * 512],
                                lhsT=xg[:, k * XAW + c * 128: k * XAW + (c + 1) * 128],
                                rhs=selw[:],
                                start=(k == 0), stop=(k == K - 1))

                    # ---------- rdenT broadcast + divide ----------
                    rdT = sp.tile([4, 128], F32, tag="rdT")
                    nc.vector.tensor_scalar(out=rdT[:], in0=denT_ps[:],
                                            scalar1=4.0, scalar2=1e-16,
                                            op0=mybir.AluOpType.mult,
                                            op1=mybir.AluOpType.max)
                    nc.vector.reciprocal(out=rdT[:], in_=rdT[:])
                    nc.sync.dma_start(
                        out=rdscr[t:t + 1, :].rearrange("a (p f) -> (a p) f", p=4),
                        in_=rdT[:])
                    rdb = dstp.tile([128, 512], F32, tag="rdb")
                    nc.sync.dma_start(out=rdb[:],
                                      in_=rdscr[t:t + 1, :].to_broadcast([128, 512]))
                    ut_sb = utp.tile([128, CC1 * 512], BF16, tag="ut")
                    for c in range(CC1):
                        nc.vector.tensor_tensor(out=ut_sb[:, c * 512:(c + 1) * 512],
                                                in0=ut_ps[:, c * 512:(c + 1) * 512],
                                                in1=rdb[:],
                                                op=mybir.AluOpType.mult)

                    # ---------- project + bias ----------
                    out_ps = psh.tile([128, C1], F32, tag="hold")
                    first = True
                    for c in range(CC1):
                        for h in range(H):
                            nc.tensor.matmul(
                                out=out_ps[:],
                                lhsT=ut_sb[:, c * 512 + h * 128: c * 512 + (h + 1) * 128],
                                rhs=W1_sb[:, c * HC1 + h * C1: c * HC1 + (h + 1) * C1],
                                start=first, stop=False)
                            first = False
                    nc.tensor.matmul(out=out_ps[:], lhsT=ones1[:], rhs=b1_sb[:],
                                     start=False, stop=True)
                    h1_sb = t2p.tile([128, C1], BF16, tag="h1")
                    nc.vector.tensor_copy(out=h1_sb[:], in_=out_ps[:])
                    if DEBUG:
                        h1f = sp.tile([128, C1], F32, tag="h1f")
                        nc.vector.tensor_copy(out=h1f[:], in_=out_ps[:])
                        nc.sync.dma_start(out=dbg_h1[t * 128:(t + 1) * 128, :],
                                          in_=h1f[:])

                    # ---------- xw2 = h1 @ [W2 | wa2] ----------
                    h1T = t2p.tile([128, C1], BF16, tag="h1T")
                    for c in range(CC2):
                        tp = psr.tile([128, 128], BF16, tag="rot")
                        nc.tensor.transpose(out=tp[:],
                                            in_=h1_sb[:, c * 128:(c + 1) * 128],
                                            identity=identb[:])
                        nc.vector.tensor_copy(out=h1T[:, c * 128:(c + 1) * 128],
                                              in_=tp[:])
                    xw2_sb = t2p.tile([128, TW2], BF16, tag="xw2")
                    for (lo, w) in ((0, 512), (512, 512), (1024, 8)):
                        xp = psr.tile([128, 512], F32, tag="rot")
                        for c in range(CC2):
                            nc.tensor.matmul(
                                out=xp[:, :w],
                                lhsT=h1T[:, c * 128:(c + 1) * 128],
                                rhs=W2c_sb[:, c * W2C + lo: c * W2C + lo + w],
                                start=(c == 0), stop=(c == CC2 - 1))
                        nc.vector.tensor_copy(out=xw2_sb[:, lo:lo + w], in_=xp[:, :w])
                    nc.sync.dma_start(out=t2_loc[t * 128:(t + 1) * 128, :],
                                      in_=xw2_sb[:])

                    if t % GT == GT - 1:
                        g = t // GT
                        nc.gpsimd.collective_compute(
                            "AllGather", mybir.AluOpType.bypass, replica_groups=RG,
                            ins=[t2_loc[g * GT * 128:(g + 1) * GT * 128, :]],
                            outs=[t2_full[g * NDEV * GT * 128:
                                          (g + 1) * NDEV * GT * 128, :]])

            if DEBUG:
                t2f = sp.tile([128, TW2], F32, tag="t2f")
                for i in range(NG * NDEV * GT):
                    t2b = sp.tile([128, TW2], BF16, tag="t2b")
                    nc.sync.dma_start(out=t2b[:],
                                      in_=t2_full[i * 128:(i + 1) * 128, :])
                    nc.vector.tensor_copy(out=t2f[:], in_=t2b[:])
                    nc.sync.dma_start(out=dbg_t2[i * 128:(i + 1) * 128, :],
                                      in_=t2f[:])

            # ================= layer 2 sweep + pooling =================
            with (
                tc.tile_pool(name="psu2", bufs=1, space="PSUM") as psu2,
                tc.tile_pool(name="psh2", bufs=2, space="PSUM") as psh2,
            ):
                for t in range(NT):
                    K = Ks[t]
                    o = offs[t]
                    rows = min(128, NPD - t * 128)

                    idx_sb = sp.tile([128, KMAX], I32, tag="idx")
                    nc.scalar.dma_start(out=idx_sb[:, :K], in_=x2idx_t[:, o:o + K])
                    ad_sb = sp.tile([128, 4], BF16, tag="ad")
                    if rows < 128:
                        nc.vector.memset(ad_sb[:], 0.0)
                    nc.scalar.dma_start(out=ad_sb[:rows, :],
                                      in_=t2_loc[t * 128:t * 128 + rows,
                                                 HC2 + 4:HC2 + 8])

                    hg = hgp.tile([128, KMAX * TW2], BF16, tag="hg")
                    for k in range(K):
                        nc.gpsimd.indirect_dma_start(
                            out=hg[:, k * TW2:(k + 1) * TW2], out_offset=None,
                            in_=t2_full[:, :],
                            in_offset=IndirectOffsetOnAxis(
                                ap=idx_sb[:, k:k + 1], axis=0))
                    hg3 = hg[:, :K * TW2].rearrange("p (k d) -> p k d", d=TW2)

                    selT = selp.tile([128, KMAX * 128], BF16, tag="selT")
                    nc.scalar.dma_start(out=selT[:, :K * 128],
                                        in_=selTN_t[:, o * 128:(o + K) * 128])
                    sel = selp.tile([128, KMAX * 128], BF16, tag="sel")
                    nc.scalar.dma_start(out=sel[:, :K * 128],
                                        in_=selN_t[:, o * 128:(o + K) * 128])

                    ade_ps = psh2.tile([128, 4 * KMAX], F32, tag="hold")
                    for k in range(K):
                        nc.tensor.matmul(out=ade_ps[:, 4 * k:4 * k + 4],
                                         lhsT=selT[:, k * 128:(k + 1) * 128],
                                         rhs=ad_sb[:],
                                         start=(k == 0), stop=(k == K - 1))
                    asg = sp.tile([128, 4 * KMAX], F32, tag="asg")
                    nc.vector.tensor_copy(out=asg[:, :4 * K].rearrange(
                        "p (k s) -> p k s", s=4),
                        in_=hg3[:, :, HC2:HC2 + 4])
                    z = sp.tile([128, 4 * KMAX], F32, tag="z")
                    nc.vector.tensor_tensor(out=z[:, :4 * K], in0=ade_ps[:, :4 * K],
                                            in1=asg[:, :4 * K],
                                            op=mybir.AluOpType.add)
                    zs = sp.tile([128, 4 * KMAX], F32, tag="zs")
                    nc.vector.tensor_scalar_mul(zs[:, :4 * K], z[:, :4 * K], NEG)
                    nc.vector.tensor_tensor(out=z[:, :4 * K], in0=z[:, :4 * K],
                                            in1=zs[:, :4 * K],
                                            op=mybir.AluOpType.max)
                    exf = sp.tile([128, 4 * KMAX], F32, tag="exf")
                    nc.scalar.activation(out=exf[:, :4 * K], in_=z[:, :4 * K],
                                         func=mybir.ActivationFunctionType.Exp)
                    ex = sp.tile([128, 4 * KMAX], BF16, tag="ex")
                    nc.vector.tensor_copy(out=ex[:, :4 * K], in_=exf[:, :4 * K])

                    # agg[d, (h,c)] += ex * xw2 ; den[d, h] += ex
                    agg_ps = psu2.tile([128, 4 * 512], F32, tag="ut2")
                    den_ps = psh2.tile([128, 4], F32, tag="hold")
                    for k in range(K):
                        selh = selwp.tile([128, 512], BF16, tag="selh")
                        for h in range(H):
                            nc.vector.tensor_scalar_mul(
                                selh[:, h * 128:(h + 1) * 128],
                                sel[:, k * 128:(k + 1) * 128],
                                exf[:, 4 * k + h:4 * k + h + 1])
                        for h in range(H):
                            nc.tensor.matmul(
                                out=agg_ps[:, h * 512:h * 512 + C2],
                                lhsT=selh[:, h * 128:(h + 1) * 128],
                                rhs=hg[:, k * TW2 + h * C2: k * TW2 + (h + 1) * C2],
                                start=(k == 0), stop=(k == K - 1))
                        nc.tensor.matmul(out=den_ps[:],
                                         lhsT=sel[:, k * 128:(k + 1) * 128],
                                         rhs=ex[:, 4 * k:4 * k + 4],
                                         start=(k == 0), stop=(k == K - 1))

                    rd2 = sp.tile([128, 4], F32, tag="rd2")
                    nc.vector.tensor_scalar(out=rd2[:], in0=den_ps[:],
                                            scalar1=4.0, scalar2=1e-16,
                                            op0=mybir.AluOpType.mult,
                                            op1=mybir.AluOpType.max)
                    nc.vector.reciprocal(out=rd2[:], in_=rd2[:])
                    h2acc = sp.tile([128, C2], F32, tag="h2acc")
                    nc.vector.tensor_scalar_mul(h2acc[:], agg_ps[:, 0:C2],
                                                rd2[:, 0:1])
                    for h in range(1, H):
                        nc.vector.scalar_tensor_tensor(
                            out=h2acc[:], in0=agg_ps[:, h * 512:h * 512 + C2],
                            scalar=rd2[:, h:h + 1], in1=h2acc[:],
                            op0=mybir.AluOpType.mult, op1=mybir.AluOpType.add)
                    h2p = t2p.tile([128, C2 + 1], BF16, tag="h2p")
                    nc.vector.tensor_tensor(out=h2p[:, :C2], in0=h2acc[:],
                                            in1=b2bc[:], op=mybir.AluOpType.add)
                    nc.vector.memset(h2p[:, C2:C2 + 1], 1.0)

                    selB = sp.tile([128, 128], BF16, tag="selB")
                    nc.vector.tensor_tensor(
                        out=selB[:],
                        in0=batchf_sb[:, t:t + 1].to_broadcast([128, 128]),
                        in1=iotaT[:], op=mybir.AluOpType.is_equal)
                    pc_ps = psh2.tile([128, C2 + 1], F32, tag="hold")
                    nc.tensor.matmul(out=pc_ps[:], lhsT=selB[:], rhs=h2p[:],
                                     start=True, stop=True)
                    nc.vector.tensor_tensor(out=poolacc[:], in0=poolacc[:],
                                            in1=pc_ps[:], op=mybir.AluOpType.add)

                # ================= pool reduce + FC =================
                nc.sync.dma_start(out=pc_loc[:, :], in_=poolacc[:])
                nc.gpsimd.collective_compute(
                    "AllReduce", mybir.AluOpType.add, replica_groups=RG,
                    ins=[pc_loc[:, :]], outs=[pc_red[:, :]])
                pc_sb = sp.tile([128, C2 + 1], F32, tag="pc")
                nc.sync.dma_start(out=pc_sb[:], in_=pc_red[:, :])
                if DEBUG:
                    nc.sync.dma_start(out=dbg_pc[:, :], in_=pc_red[:, :])
                cnt = sp.tile([128, 1], F32, tag="cnt")
                nc.vector.tensor_scalar_max(cnt[:], pc_sb[:, C2:C2 + 1], 1.0)
                nc.vector.reciprocal(out=cnt[:], in_=cnt[:])
                g_sb = sp.tile([128, C2], F32, tag="g")
                nc.vector.tensor_scalar_mul(g_sb[:], pc_sb[:, :C2], cnt[:, :1])

                y_ps = psh2.tile([128, 2], F32, tag="hold")
                for c in range(2):
                    tp = psu2.tile([128, 128], F32, tag="ut2")
                    nc.tensor.transpose(out=tp[:], in_=g_sb[:, c * 128:(c + 1) * 128],
                                        identity=ident[:])
                    gT = sp.tile([128, 128], F32, tag="gT")
                    nc.vector.tensor_copy(out=gT[:], in_=tp[:])
                    nc.tensor.matmul(out=y_ps[:], lhsT=gT[:],
                                     rhs=fcW_sb[:, 2 * c:2 * c + 2],
                                     start=(c == 0), stop=False)
                nc.tensor.matmul(out=y_ps[:], lhsT=ones1f[:], rhs=fcb_sb[:],
                                 start=False, stop=True)
                y_sb = sp.tile([128, 2], F32, tag="y")
                nc.vector.tensor_copy(out=y_sb[:], in_=y_ps[:])
                nc.sync.dma_start(out=y_t[:, :], in_=y_sb[:])

    nc.compile()
    return nc


_CACHE = {}


def kernel(**inputs):
    x = np.ascontiguousarray(np.asarray(inputs["x"], dtype=np.float32))
    edge_index = np.asarray(inputs["edge_index"])
    batch = np.asarray(inputs["batch"])
    W1 = np.asarray(inputs["W1"], dtype=np.float32)
    W2 = np.asarray(inputs["W2"], dtype=np.float32)
    a_src1 = np.asarray(inputs["a_src1"], dtype=np.float32)
    a_dst1 = np.asarray(inputs["a_dst1"], dtype=np.float32)
    a_src2 = np.asarray(inputs["a_src2"], dtype=np.float32)
    a_dst2 = np.asarray(inputs["a_dst2"], dtype=np.float32)
    b1 = np.asarray(inputs["b1"], dtype=np.float32)
    b2 = np.asarray(inputs["b2"], dtype=np.float32)
    fcW = np.ascontiguousarray(np.asarray(inputs["fcW"], dtype=np.float32))
    fcb = np.asarray(inputs["fcb"], dtype=np.float32)

    Ks, offs, SK, xidx, x2idx, selN, selTN, batchf = _host_prep(edge_index, batch)

    key = (tuple(Ks),)
    if key not in _CACHE:
        _CACHE[key] = _build(Ks, offs, SK)
    nc = _CACHE[key]

    # weight-only prep: wa = W^T a per head (folded attention projections)
    wa1 = np.zeros((C1IN, 8), dtype=np.float32)
    wa2 = np.zeros((C2IN, 8), dtype=np.float32)
    for h in range(H):
        wa1[:, h] = W1[:, h * C1:(h + 1) * C1] @ a_src1[h]
        wa1[:, 4 + h] = W1[:, h * C1:(h + 1) * C1] @ a_dst1[h]
        wa2[:, h] = W2[:, h * C2:(h + 1) * C2] @ a_src2[h]
        wa2[:, 4 + h] = W2[:, h * C2:(h + 1) * C2] @ a_dst2[h]
    W2c = np.concatenate([W2, wa2], axis=1)

    xa = np.zeros((N, XAW), dtype=BFNP)
    xa[:, :C1IN] = x.astype(BFNP)

    in_maps = []
    for d in range(NDEV):
        xloc = x[d * NPD:(d + 1) * NPD]
        in_maps.append({
            "xa": xa,
            "xlocT": np.ascontiguousarray(xloc.T).astype(BFNP),
            "W1": W1.astype(BFNP), "W2c": W2c.astype(BFNP),
            "wa1": wa1.astype(BFNP), "b1": b1.astype(BFNP), "b2": b2,
            "fcW": fcW, "fcb": fcb,
            "xidx": xidx[d], "x2idx": x2idx[d], "selN": selN[d],
            "selTN": selTN[d], "batchf": batchf[d],
        })

    import os as _os
    trace = bool(int(_os.environ.get("BASS_GAT_TRACE", "0")))
    kwargs = {}
    if trace:
        _setup_ntff_hook()
        kwargs = dict(trace=True, trace_cores=[0])
    res = run_bass_kernel_spmd(nc, in_maps, core_ids=list(range(NDEV)), **kwargs)
    if trace:
        kernel.last_exec_ns = res.exec_time_ns
        kernel.last_trace = res.instructions_and_trace
        if res.exec_time_ns is not None:
            print(f"HW exec time: {res.exec_time_ns} ns")
    if bool(int(_os.environ.get("BASS_GAT_DEBUG", "0"))):
        kernel.debug_results = res.results
    return res.results[0]["y"]


def _setup_ntff_hook():
    """The image's antenv lacks axon_hooks; synthesize it and register the
    ctypes NTFF profiling hook so trace=True works."""
    import types
    import antenv
    if hasattr(antenv, "axon_hooks"):
        return
    mod = types.ModuleType("antenv.axon_hooks")
    state = {"hook": None}
    mod.set_axon_ntff_profile_hook = lambda h: state.__setitem__("hook", h)
    mod.get_axon_ntff_profile_hook = lambda: state["hook"]
    sys.modules["antenv.axon_hooks"] = mod
    antenv.axon_hooks = mod
    try:
        from trn_agent_boot.trn_boot import _ntff_profile_via_ctypes
        hook = _ntff_profile_via_ctypes("/opt/axon/libaxon_pjrt.so")
        mod.set_axon_ntff_profile_hook(hook)
    except Exception as e:
        print("ntff hook setup failed:", e)
